# revision 4
# baseline (speedup 1.0000x reference)
"""Trainium2 Bass kernel for nn_DifferentPooling (GNN message passing).

Strategy (8 NeuronCores, SPMD):
  - Nodes padded to NP = 8*CHUNK and partitioned by node id across cores.
  - Edges partitioned by dst core; within a core, grouped into 128-node dst
    "windows". Aggregation (segment sum / softmax-sum) is done per window via
    one-hot selection matrices multiplied on the TensorEngine, accumulating in
    PSUM across the window's 128-edge tiles.
  - Feature rows are fetched with batched dma_gather (int16 indices, so the
    node table is split into two halves and each window's edges are split
    into lo/hi groups, each padded to 128-edge tiles).
  - After each layer, per-core node-feature chunks are AllGather'd so every
    core has the full table for the next layer's gathers.
  - GATv2 softmax uses exp(logit) without max subtraction (logits here are
    tiny), alpha = p / segsum(p) with a 1e-30 guard. The weighted aggregation
    uses sum_e w*eps = sum_e w*(fs+fd); since sum alpha = 1 per dst node,
    out = agg_eps/s - fd (fd subtracted only where the node has edges).
  - One-hot selection matrices (S_en and its transpose) are rebuilt on the
    Vector engine per window instead of being cached in DRAM.
  - Graph max-pooling: per-window masked-max segments interleaved into the
    last GAT layer -> small AllGather -> final max and a replicated fp32 MLP.

All biases in this problem are zeros by spec (fill="zeros"); they are not
applied on device.
"""

import sys

sys.path.insert(0, "/opt/trn_rl_repo")

import numpy as np
import ml_dtypes

bf16 = ml_dtypes.bfloat16

N_CORES = 8
P = 128  # window size / partition count
N_REAL = 50000
E_REAL = 500000
G = 64
HID = 128
HEADS = 8
DH = 16
OUT = 256


# ---------------------------------------------------------------------------
# Host-side preprocessing
# ---------------------------------------------------------------------------

def _wrap_idx(arr):
    """int idx array (len % 16 == 0) -> [128, len/16] int16 wrapped layout:
    idx i lives at [i % 16, i // 16], replicated across the 8 groups of 16
    partitions (one per Q7 core)."""
    a = np.asarray(arr, np.int16).reshape(-1, 16).T  # [16, cols]
    return np.tile(a, (8, 1))  # [128, cols]


def prep(src, dst, node2graph, nw_per_core):
    """Build per-core edge/window metadata. Returns (cfg, host arrays)."""
    NW = nw_per_core
    CHUNK = NW * P
    NP = N_CORES * CHUNK
    HALF = NP // 2
    N = len(node2graph)
    E = len(src)

    src = np.asarray(src, np.int64)
    dst = np.asarray(dst, np.int64)
    n2g = np.asarray(node2graph, np.int64)

    outdeg = np.zeros(NP, np.float32)
    np.add.at(outdeg, src, 1.0)
    indeg = np.zeros(NP, np.float32)
    np.add.at(indeg, dst, 1.0)
    ns = np.maximum(outdeg, 1.0) ** -0.5
    nd = np.maximum(indeg, 1.0) ** -0.5

    # sort edges by dst, bucket into windows
    order = np.argsort(dst, kind="stable")
    sdst = dst[order]
    ssrc = src[order]
    n_win_total = NP // P
    win_starts = np.searchsorted(sdst, np.arange(0, NP + 1, P))

    # per (global window): lo/hi edge lists sorted by src
    lo_lists, hi_lists = [], []
    max_lo = max_hi = 1
    for w in range(n_win_total):
        a, b = win_starts[w], win_starts[w + 1]
        es, ed = ssrc[a:b], sdst[a:b] - w * P
        m = es < HALF
        ordl = np.argsort(es[m], kind="stable")
        ordh = np.argsort(es[~m], kind="stable")
        lo_lists.append((es[m][ordl], ed[m][ordl]))
        hi_lists.append((es[~m][ordh] - HALF, ed[~m][ordh]))
        max_lo = max(max_lo, len(lo_lists[-1][0]))
        max_hi = max(max_hi, len(hi_lists[-1][0]))

    L = (max_lo + P - 1) // P
    H = (max_hi + P - 1) // P
    T = L + H

    # spans of SPAN_W windows (gather batching granularity)
    SPAN_W = 6 if NW >= 6 else 2
    spans = []
    w0 = 0
    while w0 < NW:
        spans.append((w0, min(SPAN_W, NW - w0)))
        w0 += SPAN_W

    per_core = []
    for c in range(N_CORES):
        idx_lo = np.zeros((NW, L * P), np.int64)
        dst_lo = np.full((NW, L * P), P, np.int64)  # sentinel 128
        idx_hi = np.zeros((NW, H * P), np.int64)
        dst_hi = np.full((NW, H * P), P, np.int64)
        for w in range(NW):
            el, dl = lo_lists[c * NW + w]
            eh, dh_ = hi_lists[c * NW + w]
            idx_lo[w, : len(el)] = el
            dst_lo[w, : len(dl)] = dl
            idx_hi[w, : len(eh)] = eh
            dst_hi[w, : len(dh_)] = dh_
        # dstloc: [NW*T, 128] -> transpose to [128, NW*T]; col w*T+t
        dstloc = np.concatenate(
            [dst_lo.reshape(NW, L, P), dst_hi.reshape(NW, H, P)], axis=1
        ).reshape(NW * T, P)
        ndw = nd[c * CHUNK : (c + 1) * CHUNK].reshape(NW, P).T.copy()
        nsw = ns[c * CHUNK : (c + 1) * CHUNK].reshape(NW, P).T.copy()
        per_core.append(
            dict(
                idx_lo=_wrap_idx(idx_lo.reshape(-1)),
                idx_hi=_wrap_idx(idx_hi.reshape(-1)),
                dstloc=np.ascontiguousarray(dstloc.T).astype(bf16),
                dstflat=np.ascontiguousarray(
                    dstloc.reshape(NW, T * P)).astype(bf16),
                ndw=np.ascontiguousarray(ndw, np.float32),
                ndnsw=np.ascontiguousarray(ndw * nsw, np.float32),
            )
        )

    # pooling segments per core: runs of equal graph id inside each window
    n2g_pad = np.full(NP, -1, np.int64)
    n2g_pad[:N] = n2g
    seg_all = []  # per core: list of (w, j0, j1, g)
    KSEG = 1
    for c in range(N_CORES):
        segs = []
        for w in range(NW):
            ids = n2g_pad[c * CHUNK + w * P : c * CHUNK + (w + 1) * P]
            j = 0
            wsegs = []
            while j < P:
                g = ids[j]
                k = j
                while k < P and ids[k] == g:
                    k += 1
                if g >= 0:
                    wsegs.append((j, k, int(g)))
                j = k
            KSEG = max(KSEG, len(wsegs))
            segs.append(wsegs)
        seg_all.append(segs)

    BIG = np.float32(1e30)
    NSEG = NW * KSEG
    for c in range(N_CORES):
        maskvec = np.full((NW, KSEG, P), -BIG, np.float32)
        gmask = np.full((G, NSEG), -BIG, np.float32)
        for w in range(NW):
            for k, (j0, j1, g) in enumerate(seg_all[c][w]):
                maskvec[w, k, j0:j1] = BIG
                gmask[g, w * KSEG + k] = BIG
        per_core[c]["poolmask"] = maskvec.reshape(NW, KSEG * P).astype(bf16)
        per_core[c]["gmask"] = gmask.astype(bf16)

    cfg = dict(NW=NW, CHUNK=CHUNK, NP=NP, HALF=HALF, L=L, H=H, T=T,
               spans=spans, KSEG=KSEG)
    return cfg, per_core, ns, nd


# ---------------------------------------------------------------------------
# Bass kernel builder
# ---------------------------------------------------------------------------

def build_nc(cfg):
    import concourse.bacc as bacc
    import concourse.bass as bass
    import concourse.mybir as mybir
    import concourse.tile as tile
    from concourse.masks import make_identity

    NW, CHUNK, NP, HALF = cfg["NW"], cfg["CHUNK"], cfg["NP"], cfg["HALF"]
    L, H, T, spans, KSEG = cfg["L"], cfg["H"], cfg["T"], cfg["spans"], cfg["KSEG"]
    FP = mybir.dt.float32
    BF = mybir.dt.bfloat16
    AO = mybir.AluOpType
    AFT = mybir.ActivationFunctionType
    GT = 6  # edge tiles per PSUM group in GAT layers

    nc = bacc.Bacc("TRN2", target_bir_lowering=False, debug=False,
                   num_devices=N_CORES)

    def din(name, shape, dt=BF):
        return nc.dram_tensor(name, shape, dt, kind="ExternalInput")

    table0 = din("table0", [NP, P])
    Wgc = [din(f"Wgc{i}", [P, P]) for i in range(2)]
    Ws = [din(f"Ws{i}", [P, P]) for i in range(3)]
    Wd = [din(f"Wd{i}", [P, P]) for i in range(3)]
    arep = [din(f"arep{i}", [P, P]) for i in range(3)]
    Wc1 = din("Wc1", [P, P], FP)
    Wc2 = din("Wc2", [P, 64], FP)
    Wc3 = din("Wc3", [64, OUT], FP)
    idx_lo = din("idx_lo", [P, NW * L * P // 16], mybir.dt.int16)
    idx_hi = din("idx_hi", [P, NW * H * P // 16], mybir.dt.int16)
    dstloc = din("dstloc", [P, NW * T])
    dstflat = din("dstflat", [NW, T * P])
    ndw = din("ndw", [P, NW], FP)
    ndnsw = din("ndnsw", [P, NW], FP)
    poolmask = din("poolmask", [NW, KSEG * P])
    gmask = din("gmask", [G, NW * KSEG])

    out_ext = nc.dram_tensor("out", [G, OUT], FP, kind="ExternalOutput")

    # internal DRAM
    agin = [nc.dram_tensor(f"agin{i}", [CHUNK, P], BF) for i in range(4)]
    tables = [table0] + [
        nc.dram_tensor(f"table{i+1}", [NP, P], BF, addr_space="Shared")
        for i in range(4)
    ]
    hgpart = nc.dram_tensor("hgpart", [P, G], FP)
    hgall = nc.dram_tensor("hgall", [N_CORES * P, G], FP, addr_space="Shared")

    RG = [list(range(N_CORES))]

    with tile.TileContext(nc) as tc:
        import contextlib

        ctx = contextlib.ExitStack()
        with ctx:
            const_pool = ctx.enter_context(tc.tile_pool(name="const", bufs=1))
            stg_pool = ctx.enter_context(tc.tile_pool(name="stg", bufs=2))
            sb_pool = ctx.enter_context(tc.tile_pool(name="sb", bufs=3))
            chunk_pool = ctx.enter_context(tc.tile_pool(name="chunk", bufs=1))
            ps_pool = ctx.enter_context(
                tc.tile_pool(name="ps", bufs=2, space="PSUM")
            )
            agg_pool = ctx.enter_context(
                tc.tile_pool(name="agg", bufs=2, space="PSUM")
            )
            mini_ps = ctx.enter_context(
                tc.tile_pool(name="minips", bufs=2, space="PSUM")
            )

            # --- constants in SBUF ---
            ident_bf = const_pool.tile([P, P], BF, tag="identbf")
            make_identity(nc, ident_bf[:])
            ident_f = const_pool.tile([P, P], FP, tag="identf")
            make_identity(nc, ident_f[:])
            iota_f = const_pool.tile([P, P], BF, tag="iota")
            iota_i = const_pool.tile([P, P], mybir.dt.int32, tag="iotai")
            nc.gpsimd.iota(iota_i[:], pattern=[[1, P]], base=0,
                           channel_multiplier=0)
            nc.vector.tensor_copy(iota_f[:], iota_i[:])
            iotap_f = const_pool.tile([P, 1], FP, tag="iotap")
            iotap_i = const_pool.tile([P, 1], mybir.dt.int32, tag="iotapi")
            nc.gpsimd.iota(iotap_i[:], pattern=[[0, 1]], base=0,
                           channel_multiplier=1)
            nc.vector.tensor_copy(iotap_f[:], iotap_i[:])

            def load_const(h, shape, dt=BF, tag=None):
                t = const_pool.tile(shape, dt, tag=tag or h.name)
                nc.sync.dma_start(t[:], h[:])
                return t

            Wgc_sb = [load_const(w, [P, P]) for w in Wgc]
            Ws_sb = [load_const(w, [P, P]) for w in Ws]
            Wd_sb = [load_const(w, [P, P]) for w in Wd]
            arep_sb = [load_const(w, [P, P]) for w in arep]
            dstloc_sb = load_const(dstloc, [P, NW * T])
            ndw_sb = load_const(ndw, [P, NW], FP)
            ndnsw_sb = load_const(ndnsw, [P, NW], FP)
            idxlo_sb = load_const(idx_lo, [P, NW * L * P // 16], mybir.dt.int16)
            idxhi_sb = load_const(idx_hi, [P, NW * H * P // 16], mybir.dt.int16)

            def s_en_build_window(w):
                """Build S_en for all T tiles of window w: [128, T, 128]
                ([e, tile, n]) on the vector engine."""
                senw = sb_pool.tile([P, T, P], BF, tag="senw")
                nc.vector.tensor_tensor(
                    out=senw[:],
                    in0=dstloc_sb[:, w * T : (w + 1) * T]
                    .unsqueeze(2)
                    .to_broadcast([P, T, P]),
                    in1=iota_f[:].unsqueeze(1).to_broadcast([P, T, P]),
                    op=AO.is_equal,
                )
                return senw

            def snt4_build(w, n4):
                """Build transposed one-hots [node, e] for windows w..w+n4."""
                dst_rep4 = sb_pool.tile(
                    [P, 4, T * P], BF, tag="dstrep4", bufs=1, name="dst_rep4"
                )
                nc.sync.dma_start(
                    dst_rep4[:, :n4, :],
                    dstflat[w : w + n4, :]
                    .unsqueeze(0)
                    .to_broadcast([P, n4, T * P]),
                )
                snT4 = sb_pool.tile([P, 4, T, P], BF, tag="snT4", bufs=1,
                                    name="snT4")
                nc.vector.tensor_scalar(
                    out=snT4[:, :n4, :, :].rearrange("p w t f -> p (w t f)"),
                    in0=dst_rep4[:, :n4, :].rearrange("p w f -> p (w f)"),
                    scalar1=iotap_f[:, 0:1],
                    scalar2=None,
                    op0=AO.is_equal,
                )
                return snT4

            def gather_span(table_l, w0, nw, transpose):
                """Gather all edges of windows [w0, w0+nw). Returns
                (stg_lo, stg_hi): transpose -> [128, 1, n] column tiles,
                else [128, ntiles, 128] row tiles."""
                nlo, nhi = nw * L * P, nw * H * P
                outs = []
                for which, n, idx_sb, colpos in (
                    ("lo", nlo, idxlo_sb, w0 * L * P),
                    ("hi", nhi, idxhi_sb, w0 * H * P),
                ):
                    half = table_l[0:HALF, :] if which == "lo" else table_l[HALF:NP, :]
                    if transpose:
                        t = stg_pool.tile([P, 1, n], BF, tag=f"stg{which}")
                        o = t[:, :, :]
                    else:
                        t = stg_pool.tile([P, n // P, P], BF, tag=f"stg{which}")
                        o = t[:, :, :]
                    nc.gpsimd.dma_gather(
                        o,
                        half,
                        idx_sb[:, colpos // 16 : (colpos + n) // 16],
                        n,
                        n,
                        P,
                        transpose=transpose,
                        single_packet=False,
                    )
                    outs.append(t)
                return outs

            # =========================================================
            # GraphConv layers
            # =========================================================
            def gc_layer(li, table_l, W_sb, agin_out, scale_sb, htag):
                hnew = chunk_pool.tile([P, NW, P], BF, tag=htag)
                for (w0, nw) in spans:
                    stg_lo, stg_hi = gather_span(table_l, w0, nw, False)
                    for wr in range(nw):
                        w = w0 + wr
                        senw = s_en_build_window(w)
                        aggT_full = agg_pool.tile([P, P + 16], FP, tag="agg",
                                                  name="aggT")
                        aggT = aggT_full[:, :P]
                        for t in range(T):
                            if t < L:
                                lhs = stg_lo[:, wr * L + t, :]
                            else:
                                lhs = stg_hi[:, wr * H + (t - L), :]
                            nc.tensor.matmul(
                                out=aggT[:],
                                lhsT=lhs,
                                rhs=senw[:, t, :],
                                start=(t == 0),
                                stop=(t == T - 1),
                            )
                        aggT_sb = sb_pool.tile([P, P], BF, tag="aggTsb")
                        nc.scalar.copy(aggT_sb[:], aggT[:])
                        op = mini_ps.tile([P, P], FP, tag="mini")
                        nc.tensor.matmul(out=op[:], lhsT=aggT_sb[:], rhs=W_sb[:],
                                         start=True, stop=True)
                        nc.scalar.activation(
                            hnew[:, w, :], op[:], AFT.Relu,
                            scale=scale_sb[:, w : w + 1],
                        )
                nc.sync.dma_start(
                    agin_out[:].rearrange("(w p) f -> p w f", p=P), hnew[:]
                )
                return hnew

            # =========================================================
            # GATv2 layers
            # =========================================================
            def fdw_prep(h_tile, Wd_l, tag):
                """fd = h @ Wd per window, from the local chunk tile."""
                fdw = chunk_pool.tile([P, NW, P], BF, tag=tag)
                for w in range(NW):
                    tp = mini_ps.tile([P, P], BF, tag="mini")
                    nc.tensor.transpose(tp[:], h_tile[:, w, :], ident_bf[:])
                    hwT = sb_pool.tile([P, P], BF, tag="hwTsb")
                    nc.scalar.copy(hwT[:], tp[:])
                    fp = mini_ps.tile([P, P], FP, tag="mini")
                    nc.tensor.matmul(out=fp[:], lhsT=hwT[:], rhs=Wd_l[:],
                                     start=True, stop=True)
                    nc.scalar.copy(fdw[:, w, :], fp[:])
                return fdw

            def gat_layer(li, table_l, hprev, fdw, Ws_l, arep_l, agin_out,
                          htag, pool_state=None):
                hnew = chunk_pool.tile([P, NW, P], BF, tag=htag)
                for (w0, nw) in spans:
                    stg_lo, stg_hi = gather_span(table_l, w0, nw, True)
                    for wr in range(nw):
                        w = w0 + wr
                        if wr % 4 == 0:
                            snT4 = snt4_build(w, min(4, nw - wr))
                        snTw = snT4[:, wr % 4]  # [P, T, P]
                        senw = s_en_build_window(w)
                        agg = agg_pool.tile([P, P + 16], FP, tag="agg")
                        for g0 in range(0, T, GT):
                            gn = min(GT, T - g0)
                            eps = ps_pool.tile([P, GT * P], FP, tag="eps")
                            for k in range(gn):
                                t = g0 + k
                                if t < L:
                                    col = (wr * L + t) * P
                                    hsT = stg_lo[:, 0, col : col + P]
                                else:
                                    col = (wr * H + (t - L)) * P
                                    hsT = stg_hi[:, 0, col : col + P]
                                sl = slice(k * P, (k + 1) * P)
                                nc.tensor.matmul(out=eps[:, sl], lhsT=hsT,
                                                 rhs=Ws_l[:], start=True,
                                                 stop=False)
                                nc.tensor.matmul(out=eps[:, sl],
                                                 lhsT=snTw[:, t, :],
                                                 rhs=fdw[:, w, :], start=False,
                                                 stop=True)
                            epsv = eps[:, : gn * P].rearrange(
                                "p (a b) -> p a b", b=P
                            )
                            elr = sb_pool.tile([P, GT, P], BF, tag="elr")
                            nc.scalar.activation(elr[:, :gn, :], epsv,
                                                 AFT.Prelu, alpha=0.2)
                            eps_sb = sb_pool.tile([P, GT, P], BF, tag="epssb")
                            nc.scalar.copy(eps_sb[:, :gn, :], epsv)
                            prod = sb_pool.tile([P, GT, P], BF, tag="prod")
                            nc.vector.tensor_tensor(
                                out=prod[:, :gn, :], in0=elr[:, :gn, :],
                                in1=arep_l[:].unsqueeze(1).to_broadcast(
                                    [P, gn, P]
                                ),
                                op=AO.mult,
                            )
                            logit = sb_pool.tile([P, GT * HEADS], FP,
                                                 tag="logit")
                            nc.vector.tensor_reduce(
                                out=logit[:, : gn * HEADS],
                                in_=prod[:, :gn, :].rearrange(
                                    "p a (h d) -> p (a h) d", d=DH
                                ),
                                axis=mybir.AxisListType.X,
                                op=AO.add,
                            )
                            wf = sb_pool.tile([P, GT, P + 8], BF, tag="wf")
                            nc.scalar.activation(
                                wf[:, :gn, P : P + 8],
                                logit[:, : gn * HEADS].rearrange(
                                    "p (a b) -> p a b", b=HEADS
                                ),
                                AFT.Exp,
                            )
                            nc.vector.tensor_tensor(
                                out=wf[:, :gn, 0:P].rearrange(
                                    "p a (h d) -> p a h d", d=DH
                                ),
                                in0=eps_sb[:, :gn, :].rearrange(
                                    "p a (h d) -> p a h d", d=DH
                                ),
                                in1=wf[:, :gn, P : P + 8]
                                .unsqueeze(3)
                                .to_broadcast([P, gn, HEADS, DH]),
                                op=AO.mult,
                            )
                            for k in range(gn):
                                t = g0 + k
                                nc.tensor.matmul(
                                    out=agg[:, : P + 8],
                                    lhsT=senw[:, t, :],
                                    rhs=wf[:, k, :],
                                    start=(t == 0),
                                    stop=(t == T - 1),
                                )
                        # ---- window flush ----
                        sguard = sb_pool.tile([P, 8], FP, tag="sguard")
                        nc.vector.tensor_scalar_max(
                            sguard[:], agg[:, P : P + 8], 1e-30
                        )
                        rec = sb_pool.tile([P, 8], FP, tag="rec")
                        nc.vector.reciprocal(rec[:], sguard[:])
                        ind = sb_pool.tile([P, 8], BF, tag="ind")
                        nc.vector.tensor_scalar(
                            out=ind[:], in0=agg[:, P : P + 8],
                            scalar1=1e-20, scalar2=None, op0=AO.is_gt,
                        )
                        fdind = sb_pool.tile([P, P], BF, tag="fdind")
                        nc.vector.tensor_tensor(
                            out=fdind[:].rearrange("p (h d) -> p h d", d=DH),
                            in0=fdw[:, w, :].rearrange("p (h d) -> p h d", d=DH),
                            in1=ind[:].unsqueeze(2).to_broadcast([P, HEADS, DH]),
                            op=AO.mult,
                        )
                        hmfd = sb_pool.tile([P, P], BF, tag="hmfd")
                        nc.vector.tensor_tensor(
                            out=hmfd[:], in0=hprev[:, w, :], in1=fdind[:],
                            op=AO.subtract,
                        )
                        o2 = sb_pool.tile([P, P], FP, tag="o2")
                        nc.vector.tensor_tensor(
                            out=o2[:].rearrange("p (h d) -> p h d", d=DH),
                            in0=agg[:, 0:P].rearrange("p (h d) -> p h d", d=DH),
                            in1=rec[:].unsqueeze(2).to_broadcast([P, HEADS, DH]),
                            op=AO.mult,
                        )
                        o3 = sb_pool.tile([P, P], FP, tag="o3")
                        nc.vector.tensor_tensor(
                            out=o3[:], in0=o2[:], in1=hmfd[:], op=AO.add
                        )
                        nc.scalar.activation(hnew[:, w, :], o3[:], AFT.Relu)
                        if pool_state is not None:
                            pool_window(pool_state, hnew, w)
                if agin_out is not None:
                    nc.sync.dma_start(
                        agin_out[:].rearrange("(w p) f -> p w f", p=P), hnew[:]
                    )
                return hnew

            # =========================================================
            # pooling (interleaved into the last GAT layer)
            # =========================================================
            def pool_window(st, hnew, w):
                tp = mini_ps.tile([P, P], BF, tag="mini")
                nc.tensor.transpose(tp[:], hnew[:, w, :], ident_bf[:])
                h5t = sb_pool.tile([P, P], BF, tag="h5t")
                nc.vector.tensor_copy(h5t[:], tp[:])
                if w % 8 == 0:
                    nw8 = min(8, NW - w)
                    st["pmask"] = sb_pool.tile(
                        [P, 8, KSEG * P], BF, tag="pmask8", bufs=1,
                        name="pmask_rep8"
                    )
                    nc.sync.dma_start(
                        st["pmask"][:, :nw8, :],
                        poolmask[w : w + nw8, :]
                        .unsqueeze(0)
                        .to_broadcast([P, nw8, KSEG * P]),
                    )
                msk = sb_pool.tile([P, KSEG, P], BF, tag="msk")
                nc.vector.tensor_tensor(
                    out=msk[:],
                    in0=h5t[:].unsqueeze(1).to_broadcast([P, KSEG, P]),
                    in1=st["pmask"][:, w % 8].rearrange(
                        "p (k b) -> p k b", b=P
                    ),
                    op=AO.min,
                )
                nc.vector.tensor_reduce(
                    out=st["stag"][:, w * KSEG : (w + 1) * KSEG],
                    in_=msk[:],
                    axis=mybir.AxisListType.X,
                    op=AO.max,
                )

            # =========================================================
            # forward pass
            # =========================================================
            h1 = gc_layer(0, tables[0], Wgc_sb[0], agin[0], ndnsw_sb, "hA")
            nc.gpsimd.collective_compute(
                "AllGather", AO.bypass, replica_groups=RG,
                ins=[agin[0].ap().opt()], outs=[tables[1].ap().opt()],
            )
            h2 = gc_layer(1, tables[1], Wgc_sb[1], agin[1], ndw_sb, "hB")
            fdw0 = fdw_prep(h2, Wd_sb[0], "fdwA")
            nc.gpsimd.collective_compute(
                "AllGather", AO.bypass, replica_groups=RG,
                ins=[agin[1].ap().opt()], outs=[tables[2].ap().opt()],
            )
            h3 = gat_layer(0, tables[2], h2, fdw0, Ws_sb[0], arep_sb[0],
                           agin[2], "hA")
            fdw1 = fdw_prep(h3, Wd_sb[1], "fdwB")
            nc.gpsimd.collective_compute(
                "AllGather", AO.bypass, replica_groups=RG,
                ins=[agin[2].ap().opt()], outs=[tables[3].ap().opt()],
            )
            h4 = gat_layer(1, tables[3], h3, fdw1, Ws_sb[1], arep_sb[1],
                           agin[3], "hB")
            fdw2 = fdw_prep(h4, Wd_sb[2], "fdwA")
            nc.gpsimd.collective_compute(
                "AllGather", AO.bypass, replica_groups=RG,
                ins=[agin[3].ap().opt()], outs=[tables[4].ap().opt()],
            )
            NSEG = NW * KSEG
            stag_t = chunk_pool.tile([P, NSEG], FP, tag="stag")
            pool_state = dict(stag=stag_t, pmask=None)
            gat_layer(2, tables[4], h4, fdw2, Ws_sb[2], arep_sb[2],
                      None, "hA", pool_state=pool_state)

            # =========================================================
            # graph-level max + MLP (replicated)
            # =========================================================
            stag = pool_state["stag"]
            gmask_all = sb_pool.tile([P, G, NSEG], BF, tag="gmaskall", bufs=1)
            nc.sync.dma_start(
                gmask_all[:],
                gmask[:].unsqueeze(0).to_broadcast([P, G, NSEG]),
            )
            gm = sb_pool.tile([P, G, NSEG], BF, tag="gm", bufs=1)
            nc.vector.tensor_tensor(
                out=gm[:],
                in0=stag[:, :NSEG].unsqueeze(1).to_broadcast([P, G, NSEG]),
                in1=gmask_all[:],
                op=AO.min,
            )
            hgT_part = sb_pool.tile([P, G], FP, tag="hgT_part")
            nc.vector.tensor_reduce(
                out=hgT_part[:], in_=gm[:],
                axis=mybir.AxisListType.X, op=AO.max,
            )
            nc.sync.dma_start(hgpart[:], hgT_part[:])
            nc.gpsimd.collective_compute(
                "AllGather", AO.bypass, replica_groups=RG,
                ins=[hgpart.ap().opt()], outs=[hgall.ap().opt()],
            )
            # final max over ranks: hgall rows = (r p)
            hgl = sb_pool.tile([P, N_CORES * G], FP, tag="hgl")
            nc.sync.dma_start(
                hgl[:].rearrange("p (r g) -> p r g", g=G),
                hgall[:].rearrange("(r p) g -> p r g", p=P),
            )
            hgT = sb_pool.tile([P, G], FP, tag="hgT")
            nc.vector.tensor_reduce(
                out=hgT[:],
                in_=hgl[:].rearrange("p (r g) -> p g r", g=G),
                axis=mybir.AxisListType.X, op=AO.max,
            )

            Wc1_sb = load_const(Wc1, [P, P], FP)
            Wc2_sb = load_const(Wc2, [P, 64], FP)
            Wc3_sb = load_const(Wc3, [64, OUT], FP)

            z1p = mini_ps.tile([G, P], FP, tag="mini")
            nc.tensor.matmul(out=z1p[:], lhsT=hgT[:], rhs=Wc1_sb[:],
                             start=True, stop=True)
            z1 = sb_pool.tile([G, P], FP, tag="z1")
            nc.scalar.activation(z1[:], z1p[:], AFT.Relu)
            z1Tp = mini_ps.tile([P, G], FP, tag="mini")
            nc.tensor.transpose(z1Tp[:], z1[:], ident_f[:G, :G])
            z1T = sb_pool.tile([P, G], FP, tag="z1T")
            nc.scalar.copy(z1T[:], z1Tp[:])
            z2p = mini_ps.tile([G, 64], FP, tag="mini")
            nc.tensor.matmul(out=z2p[:], lhsT=z1T[:], rhs=Wc2_sb[:],
                             start=True, stop=True)
            z2 = sb_pool.tile([G, 64], FP, tag="z2")
            nc.scalar.activation(z2[:], z2p[:], AFT.Relu)
            z2Tp = mini_ps.tile([64, G], FP, tag="mini")
            nc.tensor.transpose(z2Tp[:], z2[:], ident_f[:G, :G])
            z2T = sb_pool.tile([64, G], FP, tag="z2T")
            nc.scalar.copy(z2T[:], z2Tp[:])
            z3p = mini_ps.tile([G, OUT], FP, tag="mini")
            nc.tensor.matmul(out=z3p[:], lhsT=z2T[:], rhs=Wc3_sb[:],
                             start=True, stop=True)
            z3 = sb_pool.tile([G, OUT], FP, tag="z3")
            nc.scalar.copy(z3[:], z3p[:])
            nc.sync.dma_start(out_ext[:], z3[:])

    nc.compile()
    return nc


# ---------------------------------------------------------------------------
# Entry point
# ---------------------------------------------------------------------------

def _run(inputs, nw_per_core=49, trace=False):
    from concourse.bass_utils import run_bass_kernel_spmd

    src = np.asarray(inputs["src"])
    dst = np.asarray(inputs["dst"])
    n2g = np.asarray(inputs["node2graph"])
    feat = np.asarray(inputs["feature"], np.float32)

    cfg, per_core, ns, nd = prep(src, dst, n2g, nw_per_core)
    NP = cfg["NP"]

    featp = np.zeros((NP, P), np.float32)
    featp[: feat.shape[0]] = feat
    featp *= ns[:, None]
    table0 = featp.astype(bf16)

    def b(x):
        return np.ascontiguousarray(np.asarray(x, np.float32).astype(bf16))

    common = dict(
        table0=table0,
        Wgc0=b(inputs["W_gc1"]), Wgc1=b(inputs["W_gc2"]),
        Wc1=np.ascontiguousarray(np.asarray(inputs["Wc1"], np.float32)),
        Wc2=np.ascontiguousarray(np.asarray(inputs["Wc2"], np.float32)),
        Wc3=np.ascontiguousarray(np.asarray(inputs["Wc3"], np.float32)),
    )
    attn = np.asarray(inputs["attn"], np.float32)
    for i in range(3):
        common[f"Ws{i}"] = b(np.asarray(inputs["W_src"], np.float32)[i])
        common[f"Wd{i}"] = b(np.asarray(inputs["W_dst"], np.float32)[i])
        ar = np.broadcast_to(attn[i].reshape(1, HID), (P, HID))
        common[f"arep{i}"] = np.ascontiguousarray(ar).astype(bf16)

    in_maps = []
    for c in range(N_CORES):
        m = dict(common)
        m.update(per_core[c])
        in_maps.append(m)

    nc = build_nc(cfg)
    res = run_bass_kernel_spmd(nc, in_maps, core_ids=list(range(N_CORES)),
                               trace=trace)
    return np.asarray(res.results[0]["out"], np.float32), res


def kernel(**inputs) -> np.ndarray:
    out, _ = _run(inputs)
    return out


# revision 7
# speedup vs baseline: 1.0521x; 1.0521x over previous
"""Trainium2 Bass kernel for nn_DifferentPooling (GNN message passing).

Strategy (8 NeuronCores, SPMD):
  - Nodes padded to NP = 8*CHUNK and partitioned by node id across cores.
  - Edges partitioned by dst core; within a core, grouped into 128-node dst
    "windows". Aggregation (segment sum / softmax-sum) is done per window via
    one-hot selection matrices multiplied on the TensorEngine, accumulating in
    PSUM across the window's 128-edge tiles.
  - Feature rows are fetched with batched dma_gather (int16 indices, so the
    node table is split into two halves and each window's edges are split
    into lo/hi groups, each padded to 128-edge tiles).
  - After each layer, per-core node-feature chunks are AllGather'd so every
    core has the full table for the next layer's gathers.
  - GATv2 softmax uses exp(logit) without max subtraction (logits here are
    tiny), alpha = p / segsum(p) with a 1e-30 guard. The weighted aggregation
    uses sum_e w*eps = sum_e w*(fs+fd); since sum alpha = 1 per dst node,
    out = agg_eps/s - fd (fd subtracted only where the node has edges).
  - One-hot selection matrices (S_en and its transpose) are rebuilt on the
    Vector engine per window instead of being cached in DRAM.
  - Graph max-pooling: per-window masked-max segments interleaved into the
    last GAT layer -> small AllGather -> final max and a replicated fp32 MLP.

All biases in this problem are zeros by spec (fill="zeros"); they are not
applied on device.
"""

import sys

sys.path.insert(0, "/opt/trn_rl_repo")

import numpy as np
import ml_dtypes

bf16 = ml_dtypes.bfloat16

N_CORES = 8
P = 128  # window size / partition count
N_REAL = 50000
E_REAL = 500000
G = 64
HID = 128
HEADS = 8
DH = 16
OUT = 256


# ---------------------------------------------------------------------------
# Host-side preprocessing
# ---------------------------------------------------------------------------

def _wrap_idx(arr):
    """int idx array (len % 16 == 0) -> [128, len/16] int16 wrapped layout:
    idx i lives at [i % 16, i // 16], replicated across the 8 groups of 16
    partitions (one per Q7 core)."""
    a = np.asarray(arr, np.int16).reshape(-1, 16).T  # [16, cols]
    return np.tile(a, (8, 1))  # [128, cols]


def prep(src, dst, node2graph, nw_per_core):
    """Build per-core edge/window metadata. Returns (cfg, host arrays)."""
    NW = nw_per_core
    CHUNK = NW * P
    NP = N_CORES * CHUNK
    HALF = NP // 2
    N = len(node2graph)
    E = len(src)

    src = np.asarray(src, np.int64)
    dst = np.asarray(dst, np.int64)
    n2g = np.asarray(node2graph, np.int64)

    outdeg = np.zeros(NP, np.float32)
    np.add.at(outdeg, src, 1.0)
    indeg = np.zeros(NP, np.float32)
    np.add.at(indeg, dst, 1.0)
    ns = np.maximum(outdeg, 1.0) ** -0.5
    nd = np.maximum(indeg, 1.0) ** -0.5

    # sort edges by dst, bucket into windows
    order = np.argsort(dst, kind="stable")
    sdst = dst[order]
    ssrc = src[order]
    n_win_total = NP // P
    win_starts = np.searchsorted(sdst, np.arange(0, NP + 1, P))

    # per (global window): lo/hi edge lists sorted by src
    lo_lists, hi_lists = [], []
    max_lo = max_hi = 1
    for w in range(n_win_total):
        a, b = win_starts[w], win_starts[w + 1]
        es, ed = ssrc[a:b], sdst[a:b] - w * P
        m = es < HALF
        ordl = np.argsort(es[m], kind="stable")
        ordh = np.argsort(es[~m], kind="stable")
        lo_lists.append((es[m][ordl], ed[m][ordl]))
        hi_lists.append((es[~m][ordh] - HALF, ed[~m][ordh]))
        max_lo = max(max_lo, len(lo_lists[-1][0]))
        max_hi = max(max_hi, len(hi_lists[-1][0]))

    L = (max_lo + P - 1) // P
    H = (max_hi + P - 1) // P
    T = L + H

    # spans of SPAN_W windows (gather batching granularity)
    SPAN_W = 6 if NW >= 6 else 2
    spans = []
    w0 = 0
    while w0 < NW:
        spans.append((w0, min(SPAN_W, NW - w0)))
        w0 += SPAN_W

    per_core = []
    for c in range(N_CORES):
        idx_lo = np.zeros((NW, L * P), np.int64)
        dst_lo = np.full((NW, L * P), P, np.int64)  # sentinel 128
        idx_hi = np.zeros((NW, H * P), np.int64)
        dst_hi = np.full((NW, H * P), P, np.int64)
        for w in range(NW):
            el, dl = lo_lists[c * NW + w]
            eh, dh_ = hi_lists[c * NW + w]
            idx_lo[w, : len(el)] = el
            dst_lo[w, : len(dl)] = dl
            idx_hi[w, : len(eh)] = eh
            dst_hi[w, : len(dh_)] = dh_
        # dstloc: [NW*T, 128] -> transpose to [128, NW*T]; col w*T+t
        dstloc = np.concatenate(
            [dst_lo.reshape(NW, L, P), dst_hi.reshape(NW, H, P)], axis=1
        ).reshape(NW * T, P)
        ndw = nd[c * CHUNK : (c + 1) * CHUNK].reshape(NW, P).T.copy()
        nsw = ns[c * CHUNK : (c + 1) * CHUNK].reshape(NW, P).T.copy()
        per_core.append(
            dict(
                idx_lo=_wrap_idx(idx_lo.reshape(-1)),
                idx_hi=_wrap_idx(idx_hi.reshape(-1)),
                dstloc=np.ascontiguousarray(dstloc.T).astype(bf16),
                dstflat=np.ascontiguousarray(
                    dstloc.reshape(NW, T * P)).astype(bf16),
                ndw=np.ascontiguousarray(ndw, np.float32),
                ndnsw=np.ascontiguousarray(ndw * nsw, np.float32),
            )
        )

    # pooling segments per core: runs of equal graph id inside each window
    n2g_pad = np.full(NP, -1, np.int64)
    n2g_pad[:N] = n2g
    seg_all = []  # per core: list of (w, j0, j1, g)
    KSEG = 1
    for c in range(N_CORES):
        segs = []
        for w in range(NW):
            ids = n2g_pad[c * CHUNK + w * P : c * CHUNK + (w + 1) * P]
            j = 0
            wsegs = []
            while j < P:
                g = ids[j]
                k = j
                while k < P and ids[k] == g:
                    k += 1
                if g >= 0:
                    wsegs.append((j, k, int(g)))
                j = k
            KSEG = max(KSEG, len(wsegs))
            segs.append(wsegs)
        seg_all.append(segs)

    BIG = np.float32(1e30)
    NSEG = NW * KSEG
    for c in range(N_CORES):
        maskvec = np.full((NW, KSEG, P), -BIG, np.float32)
        gmask = np.full((G, NSEG), -BIG, np.float32)
        for w in range(NW):
            for k, (j0, j1, g) in enumerate(seg_all[c][w]):
                maskvec[w, k, j0:j1] = BIG
                gmask[g, w * KSEG + k] = BIG
        per_core[c]["poolmask"] = maskvec.reshape(NW, KSEG * P).astype(bf16)
        per_core[c]["gmask"] = gmask.astype(bf16)

    cfg = dict(NW=NW, CHUNK=CHUNK, NP=NP, HALF=HALF, L=L, H=H, T=T,
               spans=spans, KSEG=KSEG)
    return cfg, per_core, ns, nd


# ---------------------------------------------------------------------------
# Bass kernel builder
# ---------------------------------------------------------------------------

def build_nc(cfg):
    import concourse.bacc as bacc
    import concourse.bass as bass
    import concourse.mybir as mybir
    import concourse.tile as tile
    from concourse.masks import make_identity

    NW, CHUNK, NP, HALF = cfg["NW"], cfg["CHUNK"], cfg["NP"], cfg["HALF"]
    L, H, T, spans, KSEG = cfg["L"], cfg["H"], cfg["T"], cfg["spans"], cfg["KSEG"]
    FP = mybir.dt.float32
    BF = mybir.dt.bfloat16
    AO = mybir.AluOpType
    AFT = mybir.ActivationFunctionType
    GT = 6  # edge tiles per PSUM group in GAT layers

    nc = bacc.Bacc("TRN2", target_bir_lowering=False, debug=False,
                   num_devices=N_CORES)

    def din(name, shape, dt=BF):
        return nc.dram_tensor(name, shape, dt, kind="ExternalInput")

    table0 = din("table0", [NP, P])
    Wgc = [din(f"Wgc{i}", [P, P]) for i in range(2)]
    Ws = [din(f"Ws{i}", [P, P]) for i in range(3)]
    Wd = [din(f"Wd{i}", [P, P]) for i in range(3)]
    arep = [din(f"arep{i}", [P, P]) for i in range(3)]
    Wc1 = din("Wc1", [P, P], FP)
    Wc2 = din("Wc2", [P, 64], FP)
    Wc3 = din("Wc3", [64, OUT], FP)
    idx_lo = din("idx_lo", [P, NW * L * P // 16], mybir.dt.int16)
    idx_hi = din("idx_hi", [P, NW * H * P // 16], mybir.dt.int16)
    dstloc = din("dstloc", [P, NW * T])
    dstflat = din("dstflat", [NW, T * P])
    ndw = din("ndw", [P, NW], FP)
    ndnsw = din("ndnsw", [P, NW], FP)
    poolmask = din("poolmask", [NW, KSEG * P])
    gmask = din("gmask", [G, NW * KSEG])

    out_ext = nc.dram_tensor("out", [G, OUT], FP, kind="ExternalOutput")

    # internal DRAM
    agin = [nc.dram_tensor(f"agin{i}", [CHUNK, P], BF) for i in range(4)]
    tables = [table0] + [
        nc.dram_tensor(f"table{i+1}", [NP, P], BF, addr_space="Shared")
        for i in range(4)
    ]
    sden = nc.dram_tensor("sden", [NW, P, T * P], BF)
    hgpart = nc.dram_tensor("hgpart", [P, G], FP)
    hgall = nc.dram_tensor("hgall", [N_CORES * P, G], FP, addr_space="Shared")

    RG = [list(range(N_CORES))]

    with tile.TileContext(nc) as tc:
        import contextlib

        ctx = contextlib.ExitStack()
        with ctx:
            const_pool = ctx.enter_context(tc.tile_pool(name="const", bufs=1))
            stg_pool = ctx.enter_context(tc.tile_pool(name="stg", bufs=2))
            sb_pool = ctx.enter_context(tc.tile_pool(name="sb", bufs=3))
            chunk_pool = ctx.enter_context(tc.tile_pool(name="chunk", bufs=1))
            ps_pool = ctx.enter_context(
                tc.tile_pool(name="ps", bufs=2, space="PSUM")
            )
            agg_pool = ctx.enter_context(
                tc.tile_pool(name="agg", bufs=2, space="PSUM")
            )
            mini_ps = ctx.enter_context(
                tc.tile_pool(name="minips", bufs=2, space="PSUM")
            )

            # --- constants in SBUF ---
            ident_bf = const_pool.tile([P, P], BF, tag="identbf")
            make_identity(nc, ident_bf[:])
            ident_f = const_pool.tile([P, P], FP, tag="identf")
            make_identity(nc, ident_f[:])
            iota_f = const_pool.tile([P, P], BF, tag="iota")
            iota_i = const_pool.tile([P, P], mybir.dt.int32, tag="iotai")
            nc.gpsimd.iota(iota_i[:], pattern=[[1, P]], base=0,
                           channel_multiplier=0)
            nc.vector.tensor_copy(iota_f[:], iota_i[:])
            iotap_f = const_pool.tile([P, 1], FP, tag="iotap")
            iotap_i = const_pool.tile([P, 1], mybir.dt.int32, tag="iotapi")
            nc.gpsimd.iota(iotap_i[:], pattern=[[0, 1]], base=0,
                           channel_multiplier=1)
            nc.vector.tensor_copy(iotap_f[:], iotap_i[:])

            def load_const(h, shape, dt=BF, tag=None):
                t = const_pool.tile(shape, dt, tag=tag or h.name)
                nc.sync.dma_start(t[:], h[:])
                return t

            Wgc_sb = [load_const(w, [P, P]) for w in Wgc]
            Ws_sb = [load_const(w, [P, P]) for w in Ws]
            Wd_sb = [load_const(w, [P, P]) for w in Wd]
            arep_sb = [load_const(w, [P, P]) for w in arep]
            dstloc_sb = load_const(dstloc, [P, NW * T])
            ndw_sb = load_const(ndw, [P, NW], FP)
            ndnsw_sb = load_const(ndnsw, [P, NW], FP)
            idxlo_sb = load_const(idx_lo, [P, NW * L * P // 16], mybir.dt.int16)
            idxhi_sb = load_const(idx_hi, [P, NW * H * P // 16], mybir.dt.int16)

            def s_en_build_window(w):
                """Build S_en for all T tiles of window w: [128, T, 128]
                ([e, tile, n]) on the vector engine."""
                senw = sb_pool.tile([P, T, P], BF, tag="senw", bufs=2)
                nc.vector.tensor_tensor(
                    out=senw[:],
                    in0=dstloc_sb[:, w * T : (w + 1) * T]
                    .unsqueeze(2)
                    .to_broadcast([P, T, P]),
                    in1=iota_f[:].unsqueeze(1).to_broadcast([P, T, P]),
                    op=AO.is_equal,
                )
                return senw

            SLOAD_W = 2  # windows per S_en reload DMA

            def sden_prep():
                for w in range(NW):
                    senw = s_en_build_window(w)
                    nc.sync.dma_start(
                        sden[w].rearrange("p f -> p f"),
                        senw[:].rearrange("p t f -> p (t f)"),
                    )

            def s_en_load(w0, nwin):
                sload = sb_pool.tile([P, SLOAD_W, T * P], BF, tag="sload",
                                     bufs=2)
                nc.sync.dma_start(
                    sload[:, :nwin, :],
                    sden[w0 : w0 + nwin].rearrange("w p f -> p w f"),
                )
                return sload

            def snt4_build(w, n4):
                """Build transposed one-hots [node, e] for windows w..w+n4."""
                dst_rep4 = sb_pool.tile(
                    [P, 4, T * P], BF, tag="dstrep4", bufs=1, name="dst_rep4"
                )
                nc.sync.dma_start(
                    dst_rep4[:, :n4, :],
                    dstflat[w : w + n4, :]
                    .unsqueeze(0)
                    .to_broadcast([P, n4, T * P]),
                )
                snT4 = sb_pool.tile([P, 4, T, P], BF, tag="snT4", bufs=1,
                                    name="snT4")
                nc.vector.tensor_scalar(
                    out=snT4[:, :n4, :, :].rearrange("p w t f -> p (w t f)"),
                    in0=dst_rep4[:, :n4, :].rearrange("p w f -> p (w f)"),
                    scalar1=iotap_f[:, 0:1],
                    scalar2=None,
                    op0=AO.is_equal,
                )
                return snT4

            def gather_span(table_l, w0, nw, transpose):
                """Gather all edges of windows [w0, w0+nw). Returns
                (stg_lo, stg_hi): transpose -> [128, 1, n] column tiles,
                else [128, ntiles, 128] row tiles."""
                nlo, nhi = nw * L * P, nw * H * P
                outs = []
                for which, n, idx_sb, colpos in (
                    ("lo", nlo, idxlo_sb, w0 * L * P),
                    ("hi", nhi, idxhi_sb, w0 * H * P),
                ):
                    half = table_l[0:HALF, :] if which == "lo" else table_l[HALF:NP, :]
                    if transpose:
                        t = stg_pool.tile([P, 1, n], BF, tag=f"stg{which}")
                        o = t[:, :, :]
                    else:
                        t = stg_pool.tile([P, n // P, P], BF, tag=f"stg{which}")
                        o = t[:, :, :]
                    nc.gpsimd.dma_gather(
                        o,
                        half,
                        idx_sb[:, colpos // 16 : (colpos + n) // 16],
                        n,
                        n,
                        P,
                        transpose=transpose,
                        single_packet=False,
                    )
                    outs.append(t)
                return outs

            # =========================================================
            # GraphConv layers
            # =========================================================
            def gc_layer(li, table_l, W_sb, agin_out, scale_sb, htag):
                hnew = chunk_pool.tile([P, NW, P], BF, tag=htag)
                for (w0, nw) in spans:
                    stg_lo, stg_hi = gather_span(table_l, w0, nw, False)
                    for wr in range(nw):
                        w = w0 + wr
                        senw = s_en_build_window(w)
                        aggT_full = agg_pool.tile([P, P + 16], FP, tag="agg",
                                                  name="aggT")
                        aggT = aggT_full[:, :P]
                        for t in range(T):
                            if t < L:
                                lhs = stg_lo[:, wr * L + t, :]
                            else:
                                lhs = stg_hi[:, wr * H + (t - L), :]
                            nc.tensor.matmul(
                                out=aggT[:],
                                lhsT=lhs,
                                rhs=senw[:, t, :],
                                start=(t == 0),
                                stop=(t == T - 1),
                            )
                        aggT_sb = sb_pool.tile([P, P], BF, tag="aggTsb")
                        nc.scalar.copy(aggT_sb[:], aggT[:])
                        op = mini_ps.tile([P, P], FP, tag="mini")
                        nc.tensor.matmul(out=op[:], lhsT=aggT_sb[:], rhs=W_sb[:],
                                         start=True, stop=True)
                        nc.scalar.activation(
                            hnew[:, w, :], op[:], AFT.Relu,
                            scale=scale_sb[:, w : w + 1],
                        )
                nc.sync.dma_start(
                    agin_out[:].rearrange("(w p) f -> p w f", p=P), hnew[:]
                )
                return hnew

            # =========================================================
            # GATv2 layers
            # =========================================================
            def fdw_prep(h_tile, Wd_l, tag):
                """fd = h @ Wd per window, from the local chunk tile."""
                fdw = chunk_pool.tile([P, NW, P], BF, tag=tag)
                for w in range(NW):
                    tp = mini_ps.tile([P, P], BF, tag="mini")
                    nc.tensor.transpose(tp[:], h_tile[:, w, :], ident_bf[:])
                    hwT = sb_pool.tile([P, P], BF, tag="hwTsb")
                    nc.scalar.copy(hwT[:], tp[:])
                    fp = mini_ps.tile([P, P], FP, tag="mini")
                    nc.tensor.matmul(out=fp[:], lhsT=hwT[:], rhs=Wd_l[:],
                                     start=True, stop=True)
                    nc.scalar.copy(fdw[:, w, :], fp[:])
                return fdw

            def gat_layer(li, table_l, hprev, fdw, Ws_l, arep_l, agin_out,
                          htag, pool_state=None):
                hnew = chunk_pool.tile([P, NW, P], BF, tag=htag)
                for (w0, nw) in spans:
                    stg_lo, stg_hi = gather_span(table_l, w0, nw, True)
                    for wr in range(nw):
                        w = w0 + wr
                        if wr % 4 == 0:
                            snT4 = snt4_build(w, min(4, nw - wr))
                        if wr % SLOAD_W == 0:
                            sload = s_en_load(w, min(SLOAD_W, nw - wr))
                        snTw = snT4[:, wr % 4]  # [P, T, P]
                        swi = wr % SLOAD_W
                        agg = agg_pool.tile([P, P + 16], FP, tag="agg")
                        for g0 in range(0, T, GT):
                            gn = min(GT, T - g0)
                            eps = ps_pool.tile([P, GT * P], FP, tag="eps")
                            for k in range(gn):
                                t = g0 + k
                                if t < L:
                                    col = (wr * L + t) * P
                                    hsT = stg_lo[:, 0, col : col + P]
                                else:
                                    col = (wr * H + (t - L)) * P
                                    hsT = stg_hi[:, 0, col : col + P]
                                sl = slice(k * P, (k + 1) * P)
                                nc.tensor.matmul(out=eps[:, sl], lhsT=hsT,
                                                 rhs=Ws_l[:], start=True,
                                                 stop=False)
                                nc.tensor.matmul(out=eps[:, sl],
                                                 lhsT=snTw[:, t, :],
                                                 rhs=fdw[:, w, :], start=False,
                                                 stop=True)
                            epsv = eps[:, : gn * P].rearrange(
                                "p (a b) -> p a b", b=P
                            )
                            elr = sb_pool.tile([P, GT, P], BF, tag="elr",
                                               bufs=2)
                            nc.scalar.activation(elr[:, :gn, :], epsv,
                                                 AFT.Prelu, alpha=0.2)
                            prod = sb_pool.tile([P, GT, P], BF, tag="prod",
                                                bufs=2)
                            nc.vector.tensor_tensor(
                                out=prod[:, :gn, :], in0=elr[:, :gn, :],
                                in1=arep_l[:].unsqueeze(1).to_broadcast(
                                    [P, gn, P]
                                ),
                                op=AO.mult,
                            )
                            pv = prod[:, :gn, :].rearrange(
                                "p a (h d) -> p (a h) d", d=DH
                            )
                            fold8 = sb_pool.tile([P, GT * HEADS, 8], BF,
                                                 tag="fold8", bufs=2)
                            nc.vector.tensor_tensor(
                                out=fold8[:, : gn * HEADS, :],
                                in0=pv[:, :, 0:8], in1=pv[:, :, 8:16],
                                op=AO.add,
                            )
                            logit = sb_pool.tile([P, GT * HEADS], FP,
                                                 tag="logit")
                            nc.vector.tensor_reduce(
                                out=logit[:, : gn * HEADS],
                                in_=fold8[:, : gn * HEADS, :],
                                axis=mybir.AxisListType.X,
                                op=AO.add,
                            )
                            wf = sb_pool.tile([P, GT, P + 8], BF, tag="wf",
                                              bufs=2)
                            nc.scalar.activation(
                                wf[:, :gn, P : P + 8],
                                logit[:, : gn * HEADS].rearrange(
                                    "p (a b) -> p a b", b=HEADS
                                ),
                                AFT.Exp,
                            )
                            nc.vector.tensor_tensor(
                                out=wf[:, :gn, 0:P].rearrange(
                                    "p a (h d) -> p a h d", d=DH
                                ),
                                in0=eps[:, : gn * P].rearrange(
                                    "p (a h d) -> p a h d", h=HEADS, d=DH
                                ),
                                in1=wf[:, :gn, P : P + 8]
                                .unsqueeze(3)
                                .to_broadcast([P, gn, HEADS, DH]),
                                op=AO.mult,
                            )
                            for k in range(gn):
                                t = g0 + k
                                nc.tensor.matmul(
                                    out=agg[:, : P + 8],
                                    lhsT=sload[:, swi, t * P : (t + 1) * P],
                                    rhs=wf[:, k, :],
                                    start=(t == 0),
                                    stop=(t == T - 1),
                                )
                        # ---- window flush ----
                        sguard = sb_pool.tile([P, 8], FP, tag="sguard")
                        nc.vector.tensor_scalar_max(
                            sguard[:], agg[:, P : P + 8], 1e-30
                        )
                        rec = sb_pool.tile([P, 8], FP, tag="rec")
                        nc.vector.reciprocal(rec[:], sguard[:])
                        ind = sb_pool.tile([P, 8], BF, tag="ind")
                        nc.vector.tensor_scalar(
                            out=ind[:], in0=agg[:, P : P + 8],
                            scalar1=1e-20, scalar2=None, op0=AO.is_gt,
                        )
                        fdind = sb_pool.tile([P, P], BF, tag="fdind")
                        nc.gpsimd.tensor_tensor(
                            out=fdind[:].rearrange("p (h d) -> p h d", d=DH),
                            in0=fdw[:, w, :].rearrange("p (h d) -> p h d", d=DH),
                            in1=ind[:].unsqueeze(2).to_broadcast([P, HEADS, DH]),
                            op=AO.mult,
                        )
                        hmfd = sb_pool.tile([P, P], BF, tag="hmfd")
                        nc.gpsimd.tensor_tensor(
                            out=hmfd[:], in0=hprev[:, w, :], in1=fdind[:],
                            op=AO.subtract,
                        )
                        o2 = sb_pool.tile([P, P], FP, tag="o2")
                        nc.vector.tensor_tensor(
                            out=o2[:].rearrange("p (h d) -> p h d", d=DH),
                            in0=agg[:, 0:P].rearrange("p (h d) -> p h d", d=DH),
                            in1=rec[:].unsqueeze(2).to_broadcast([P, HEADS, DH]),
                            op=AO.mult,
                        )
                        o3 = sb_pool.tile([P, P], FP, tag="o3")
                        nc.gpsimd.tensor_tensor(
                            out=o3[:], in0=o2[:], in1=hmfd[:], op=AO.add
                        )
                        nc.scalar.activation(hnew[:, w, :], o3[:], AFT.Relu)
                        if pool_state is not None:
                            pool_window(pool_state, hnew, w)
                if agin_out is not None:
                    nc.sync.dma_start(
                        agin_out[:].rearrange("(w p) f -> p w f", p=P), hnew[:]
                    )
                return hnew

            # =========================================================
            # pooling (interleaved into the last GAT layer)
            # =========================================================
            def pool_window(st, hnew, w):
                tp = mini_ps.tile([P, P], BF, tag="mini")
                nc.tensor.transpose(tp[:], hnew[:, w, :], ident_bf[:])
                h5t = sb_pool.tile([P, P], BF, tag="h5t")
                nc.vector.tensor_copy(h5t[:], tp[:])
                if w % 8 == 0:
                    nw8 = min(8, NW - w)
                    st["pmask"] = sb_pool.tile(
                        [P, 8, KSEG * P], BF, tag="pmask8", bufs=1,
                        name="pmask_rep8"
                    )
                    nc.sync.dma_start(
                        st["pmask"][:, :nw8, :],
                        poolmask[w : w + nw8, :]
                        .unsqueeze(0)
                        .to_broadcast([P, nw8, KSEG * P]),
                    )
                msk = sb_pool.tile([P, KSEG, P], BF, tag="msk")
                nc.vector.tensor_tensor(
                    out=msk[:],
                    in0=h5t[:].unsqueeze(1).to_broadcast([P, KSEG, P]),
                    in1=st["pmask"][:, w % 8].rearrange(
                        "p (k b) -> p k b", b=P
                    ),
                    op=AO.min,
                )
                nc.vector.tensor_reduce(
                    out=st["stag"][:, w * KSEG : (w + 1) * KSEG],
                    in_=msk[:],
                    axis=mybir.AxisListType.X,
                    op=AO.max,
                )

            # =========================================================
            # forward pass
            # =========================================================
            h1 = gc_layer(0, tables[0], Wgc_sb[0], agin[0], ndnsw_sb, "hA")
            nc.gpsimd.collective_compute(
                "AllGather", AO.bypass, replica_groups=RG,
                ins=[agin[0].ap().opt()], outs=[tables[1].ap().opt()],
            )
            sden_prep()
            h2 = gc_layer(1, tables[1], Wgc_sb[1], agin[1], ndw_sb, "hB")
            fdw0 = fdw_prep(h2, Wd_sb[0], "fdwA")
            nc.gpsimd.collective_compute(
                "AllGather", AO.bypass, replica_groups=RG,
                ins=[agin[1].ap().opt()], outs=[tables[2].ap().opt()],
            )
            h3 = gat_layer(0, tables[2], h2, fdw0, Ws_sb[0], arep_sb[0],
                           agin[2], "hA")
            fdw1 = fdw_prep(h3, Wd_sb[1], "fdwB")
            nc.gpsimd.collective_compute(
                "AllGather", AO.bypass, replica_groups=RG,
                ins=[agin[2].ap().opt()], outs=[tables[3].ap().opt()],
            )
            h4 = gat_layer(1, tables[3], h3, fdw1, Ws_sb[1], arep_sb[1],
                           agin[3], "hB")
            fdw2 = fdw_prep(h4, Wd_sb[2], "fdwA")
            nc.gpsimd.collective_compute(
                "AllGather", AO.bypass, replica_groups=RG,
                ins=[agin[3].ap().opt()], outs=[tables[4].ap().opt()],
            )
            NSEG = NW * KSEG
            stag_t = chunk_pool.tile([P, NSEG], FP, tag="stag")
            pool_state = dict(stag=stag_t, pmask=None)
            gat_layer(2, tables[4], h4, fdw2, Ws_sb[2], arep_sb[2],
                      None, "hA", pool_state=pool_state)

            # =========================================================
            # graph-level max + MLP (replicated)
            # =========================================================
            stag = pool_state["stag"]
            gmask_all = sb_pool.tile([P, G, NSEG], BF, tag="gmaskall", bufs=1)
            nc.sync.dma_start(
                gmask_all[:],
                gmask[:].unsqueeze(0).to_broadcast([P, G, NSEG]),
            )
            gm = sb_pool.tile([P, G, NSEG], BF, tag="gm", bufs=1)
            nc.vector.tensor_tensor(
                out=gm[:],
                in0=stag[:, :NSEG].unsqueeze(1).to_broadcast([P, G, NSEG]),
                in1=gmask_all[:],
                op=AO.min,
            )
            hgT_part = sb_pool.tile([P, G], FP, tag="hgT_part")
            nc.vector.tensor_reduce(
                out=hgT_part[:], in_=gm[:],
                axis=mybir.AxisListType.X, op=AO.max,
            )
            nc.sync.dma_start(hgpart[:], hgT_part[:])
            nc.gpsimd.collective_compute(
                "AllGather", AO.bypass, replica_groups=RG,
                ins=[hgpart.ap().opt()], outs=[hgall.ap().opt()],
            )
            # final max over ranks: hgall rows = (r p)
            hgl = sb_pool.tile([P, N_CORES * G], FP, tag="hgl")
            nc.sync.dma_start(
                hgl[:].rearrange("p (r g) -> p r g", g=G),
                hgall[:].rearrange("(r p) g -> p r g", p=P),
            )
            hgT = sb_pool.tile([P, G], FP, tag="hgT")
            nc.vector.tensor_reduce(
                out=hgT[:],
                in_=hgl[:].rearrange("p (r g) -> p g r", g=G),
                axis=mybir.AxisListType.X, op=AO.max,
            )

            Wc1_sb = load_const(Wc1, [P, P], FP)
            Wc2_sb = load_const(Wc2, [P, 64], FP)
            Wc3_sb = load_const(Wc3, [64, OUT], FP)

            z1p = mini_ps.tile([G, P], FP, tag="mini")
            nc.tensor.matmul(out=z1p[:], lhsT=hgT[:], rhs=Wc1_sb[:],
                             start=True, stop=True)
            z1 = sb_pool.tile([G, P], FP, tag="z1")
            nc.scalar.activation(z1[:], z1p[:], AFT.Relu)
            z1Tp = mini_ps.tile([P, G], FP, tag="mini")
            nc.tensor.transpose(z1Tp[:], z1[:], ident_f[:G, :G])
            z1T = sb_pool.tile([P, G], FP, tag="z1T")
            nc.scalar.copy(z1T[:], z1Tp[:])
            z2p = mini_ps.tile([G, 64], FP, tag="mini")
            nc.tensor.matmul(out=z2p[:], lhsT=z1T[:], rhs=Wc2_sb[:],
                             start=True, stop=True)
            z2 = sb_pool.tile([G, 64], FP, tag="z2")
            nc.scalar.activation(z2[:], z2p[:], AFT.Relu)
            z2Tp = mini_ps.tile([64, G], FP, tag="mini")
            nc.tensor.transpose(z2Tp[:], z2[:], ident_f[:G, :G])
            z2T = sb_pool.tile([64, G], FP, tag="z2T")
            nc.scalar.copy(z2T[:], z2Tp[:])
            z3p = mini_ps.tile([G, OUT], FP, tag="mini")
            nc.tensor.matmul(out=z3p[:], lhsT=z2T[:], rhs=Wc3_sb[:],
                             start=True, stop=True)
            z3 = sb_pool.tile([G, OUT], FP, tag="z3")
            nc.scalar.copy(z3[:], z3p[:])
            nc.sync.dma_start(out_ext[:], z3[:])

    nc.compile()
    return nc


# ---------------------------------------------------------------------------
# Entry point
# ---------------------------------------------------------------------------

def _run(inputs, nw_per_core=49, trace=False):
    from concourse.bass_utils import run_bass_kernel_spmd

    src = np.asarray(inputs["src"])
    dst = np.asarray(inputs["dst"])
    n2g = np.asarray(inputs["node2graph"])
    feat = np.asarray(inputs["feature"], np.float32)

    cfg, per_core, ns, nd = prep(src, dst, n2g, nw_per_core)
    NP = cfg["NP"]

    featp = np.zeros((NP, P), np.float32)
    featp[: feat.shape[0]] = feat
    featp *= ns[:, None]
    table0 = featp.astype(bf16)

    def b(x):
        return np.ascontiguousarray(np.asarray(x, np.float32).astype(bf16))

    common = dict(
        table0=table0,
        Wgc0=b(inputs["W_gc1"]), Wgc1=b(inputs["W_gc2"]),
        Wc1=np.ascontiguousarray(np.asarray(inputs["Wc1"], np.float32)),
        Wc2=np.ascontiguousarray(np.asarray(inputs["Wc2"], np.float32)),
        Wc3=np.ascontiguousarray(np.asarray(inputs["Wc3"], np.float32)),
    )
    attn = np.asarray(inputs["attn"], np.float32)
    for i in range(3):
        common[f"Ws{i}"] = b(np.asarray(inputs["W_src"], np.float32)[i])
        common[f"Wd{i}"] = b(np.asarray(inputs["W_dst"], np.float32)[i])
        ar = np.broadcast_to(attn[i].reshape(1, HID), (P, HID))
        common[f"arep{i}"] = np.ascontiguousarray(ar).astype(bf16)

    in_maps = []
    for c in range(N_CORES):
        m = dict(common)
        m.update(per_core[c])
        in_maps.append(m)

    nc = build_nc(cfg)
    res = run_bass_kernel_spmd(nc, in_maps, core_ids=list(range(N_CORES)),
                               trace=trace)
    return np.asarray(res.results[0]["out"], np.float32), res


def kernel(**inputs) -> np.ndarray:
    out, _ = _run(inputs)
    return out


# revision 9
# speedup vs baseline: 1.0526x; 1.0006x over previous
"""Trainium2 Bass kernel for nn_DifferentPooling (GNN message passing).

Strategy (8 NeuronCores, SPMD):
  - Nodes padded to NP = 8*CHUNK and partitioned by node id across cores.
  - Edges partitioned by dst core; within a core, grouped into 128-node dst
    "windows". Aggregation (segment sum / softmax-sum) is done per window via
    one-hot selection matrices multiplied on the TensorEngine, accumulating in
    PSUM across the window's 128-edge tiles.
  - Feature rows are fetched with batched dma_gather (int16 indices, so the
    node table is split into two halves and each window's edges are split
    into lo/hi groups, each padded to 128-edge tiles).
  - After each layer, per-core node-feature chunks are AllGather'd so every
    core has the full table for the next layer's gathers.
  - GATv2 softmax uses exp(logit) without max subtraction (logits here are
    tiny), alpha = p / segsum(p) with a 1e-30 guard. The weighted aggregation
    uses sum_e w*eps = sum_e w*(fs+fd); since sum alpha = 1 per dst node,
    out = agg_eps/s - fd (fd subtracted only where the node has edges).
  - One-hot selection matrices (S_en and its transpose) are rebuilt on the
    Vector engine per window instead of being cached in DRAM.
  - Graph max-pooling: per-window masked-max segments interleaved into the
    last GAT layer -> small AllGather -> final max and a replicated fp32 MLP.

All biases in this problem are zeros by spec (fill="zeros"); they are not
applied on device.
"""

import sys

sys.path.insert(0, "/opt/trn_rl_repo")

import numpy as np
import ml_dtypes

bf16 = ml_dtypes.bfloat16

N_CORES = 8
P = 128  # window size / partition count
N_REAL = 50000
E_REAL = 500000
G = 64
HID = 128
HEADS = 8
DH = 16
OUT = 256


# ---------------------------------------------------------------------------
# Host-side preprocessing
# ---------------------------------------------------------------------------

def _wrap_idx(arr):
    """int idx array (len % 16 == 0) -> [128, len/16] int16 wrapped layout:
    idx i lives at [i % 16, i // 16], replicated across the 8 groups of 16
    partitions (one per Q7 core)."""
    a = np.asarray(arr, np.int16).reshape(-1, 16).T  # [16, cols]
    return np.tile(a, (8, 1))  # [128, cols]


def prep(src, dst, node2graph, nw_per_core):
    """Build per-core edge/window metadata. Returns (cfg, host arrays)."""
    NW = nw_per_core
    CHUNK = NW * P
    NP = N_CORES * CHUNK
    HALF = NP // 2
    N = len(node2graph)
    E = len(src)

    src = np.asarray(src, np.int64)
    dst = np.asarray(dst, np.int64)
    n2g = np.asarray(node2graph, np.int64)

    outdeg = np.zeros(NP, np.float32)
    np.add.at(outdeg, src, 1.0)
    indeg = np.zeros(NP, np.float32)
    np.add.at(indeg, dst, 1.0)
    ns = np.maximum(outdeg, 1.0) ** -0.5
    nd = np.maximum(indeg, 1.0) ** -0.5

    # sort edges by dst, bucket into windows
    order = np.argsort(dst, kind="stable")
    sdst = dst[order]
    ssrc = src[order]
    n_win_total = NP // P
    win_starts = np.searchsorted(sdst, np.arange(0, NP + 1, P))

    # per (global window): lo/hi edge lists sorted by src
    lo_lists, hi_lists = [], []
    max_lo = max_hi = 1
    for w in range(n_win_total):
        a, b = win_starts[w], win_starts[w + 1]
        es, ed = ssrc[a:b], sdst[a:b] - w * P
        m = es < HALF
        ordl = np.argsort(es[m], kind="stable")
        ordh = np.argsort(es[~m], kind="stable")
        lo_lists.append((es[m][ordl], ed[m][ordl]))
        hi_lists.append((es[~m][ordh] - HALF, ed[~m][ordh]))
        max_lo = max(max_lo, len(lo_lists[-1][0]))
        max_hi = max(max_hi, len(hi_lists[-1][0]))

    L = (max_lo + P - 1) // P
    H = (max_hi + P - 1) // P
    T = L + H

    # spans of SPAN_W windows (gather batching granularity)
    SPAN_W = 6 if NW >= 6 else 2
    spans = []
    w0 = 0
    while w0 < NW:
        spans.append((w0, min(SPAN_W, NW - w0)))
        w0 += SPAN_W

    per_core = []
    for c in range(N_CORES):
        idx_lo = np.zeros((NW, L * P), np.int64)
        dst_lo = np.full((NW, L * P), P, np.int64)  # sentinel 128
        idx_hi = np.zeros((NW, H * P), np.int64)
        dst_hi = np.full((NW, H * P), P, np.int64)
        for w in range(NW):
            el, dl = lo_lists[c * NW + w]
            eh, dh_ = hi_lists[c * NW + w]
            idx_lo[w, : len(el)] = el
            dst_lo[w, : len(dl)] = dl
            idx_hi[w, : len(eh)] = eh
            dst_hi[w, : len(dh_)] = dh_
        # dstloc: [NW*T, 128] -> transpose to [128, NW*T]; col w*T+t
        dstloc = np.concatenate(
            [dst_lo.reshape(NW, L, P), dst_hi.reshape(NW, H, P)], axis=1
        ).reshape(NW * T, P)
        ndw = nd[c * CHUNK : (c + 1) * CHUNK].reshape(NW, P).T.copy()
        nsw = ns[c * CHUNK : (c + 1) * CHUNK].reshape(NW, P).T.copy()
        per_core.append(
            dict(
                idx_lo=_wrap_idx(idx_lo.reshape(-1)),
                idx_hi=_wrap_idx(idx_hi.reshape(-1)),
                dstloc=np.ascontiguousarray(dstloc.T).astype(bf16),
                dstflat=np.ascontiguousarray(
                    dstloc.reshape(NW, T * P)).astype(bf16),
                ndw=np.ascontiguousarray(ndw, np.float32),
                ndnsw=np.ascontiguousarray(ndw * nsw, np.float32),
            )
        )

    # pooling segments per core: runs of equal graph id inside each window
    n2g_pad = np.full(NP, -1, np.int64)
    n2g_pad[:N] = n2g
    seg_all = []  # per core: list of (w, j0, j1, g)
    KSEG = 1
    for c in range(N_CORES):
        segs = []
        for w in range(NW):
            ids = n2g_pad[c * CHUNK + w * P : c * CHUNK + (w + 1) * P]
            j = 0
            wsegs = []
            while j < P:
                g = ids[j]
                k = j
                while k < P and ids[k] == g:
                    k += 1
                if g >= 0:
                    wsegs.append((j, k, int(g)))
                j = k
            KSEG = max(KSEG, len(wsegs))
            segs.append(wsegs)
        seg_all.append(segs)

    BIG = np.float32(1e30)
    NSEG = NW * KSEG
    for c in range(N_CORES):
        maskvec = np.full((NW, KSEG, P), -BIG, np.float32)
        gmask = np.full((G, NSEG), -BIG, np.float32)
        for w in range(NW):
            for k, (j0, j1, g) in enumerate(seg_all[c][w]):
                maskvec[w, k, j0:j1] = BIG
                gmask[g, w * KSEG + k] = BIG
        per_core[c]["poolmask"] = maskvec.reshape(NW, KSEG * P).astype(bf16)
        per_core[c]["gmask"] = gmask.astype(bf16)

    cfg = dict(NW=NW, CHUNK=CHUNK, NP=NP, HALF=HALF, L=L, H=H, T=T,
               spans=spans, KSEG=KSEG)
    return cfg, per_core, ns, nd


# ---------------------------------------------------------------------------
# Bass kernel builder
# ---------------------------------------------------------------------------

def build_nc(cfg):
    import concourse.bacc as bacc
    import concourse.bass as bass
    import concourse.mybir as mybir
    import concourse.tile as tile
    from concourse.masks import make_identity

    NW, CHUNK, NP, HALF = cfg["NW"], cfg["CHUNK"], cfg["NP"], cfg["HALF"]
    L, H, T, spans, KSEG = cfg["L"], cfg["H"], cfg["T"], cfg["spans"], cfg["KSEG"]
    FP = mybir.dt.float32
    BF = mybir.dt.bfloat16
    F8 = mybir.dt.float8e4
    AO = mybir.AluOpType
    AFT = mybir.ActivationFunctionType
    GT = 6  # edge tiles per PSUM group in GAT layers

    nc = bacc.Bacc("TRN2", target_bir_lowering=False, debug=False,
                   num_devices=N_CORES)

    def din(name, shape, dt=BF):
        return nc.dram_tensor(name, shape, dt, kind="ExternalInput")

    table0 = din("table0", [NP, P])
    Wgc = [din(f"Wgc{i}", [P, P]) for i in range(2)]
    Ws = [din(f"Ws{i}", [P, P]) for i in range(3)]
    Wd = [din(f"Wd{i}", [P, P]) for i in range(3)]
    arep = [din(f"arep{i}", [P, P]) for i in range(3)]
    Wc1 = din("Wc1", [P, P], FP)
    Wc2 = din("Wc2", [P, 64], FP)
    Wc3 = din("Wc3", [64, OUT], FP)
    idx_lo = din("idx_lo", [P, NW * L * P // 16], mybir.dt.int16)
    idx_hi = din("idx_hi", [P, NW * H * P // 16], mybir.dt.int16)
    dstloc = din("dstloc", [P, NW * T])
    dstflat = din("dstflat", [NW, T * P])
    ndw = din("ndw", [P, NW], FP)
    ndnsw = din("ndnsw", [P, NW], FP)
    poolmask = din("poolmask", [NW, KSEG * P])
    gmask = din("gmask", [G, NW * KSEG])

    out_ext = nc.dram_tensor("out", [G, OUT], FP, kind="ExternalOutput")

    # internal DRAM
    agin = [nc.dram_tensor(f"agin{i}", [CHUNK, P], BF) for i in range(4)]
    tables = [table0] + [
        nc.dram_tensor(f"table{i+1}", [NP, P], BF, addr_space="Shared")
        for i in range(4)
    ]
    sden = nc.dram_tensor("sden", [NW, P, T * P], mybir.dt.float8e4)
    hgpart = nc.dram_tensor("hgpart", [P, G], FP)
    hgall = nc.dram_tensor("hgall", [N_CORES * P, G], FP, addr_space="Shared")

    RG = [list(range(N_CORES))]

    with tile.TileContext(nc) as tc:
        import contextlib

        ctx = contextlib.ExitStack()
        with ctx:
            const_pool = ctx.enter_context(tc.tile_pool(name="const", bufs=1))
            stg_pool = ctx.enter_context(tc.tile_pool(name="stg", bufs=2))
            sb_pool = ctx.enter_context(tc.tile_pool(name="sb", bufs=3))
            chunk_pool = ctx.enter_context(tc.tile_pool(name="chunk", bufs=1))
            ps_pool = ctx.enter_context(
                tc.tile_pool(name="ps", bufs=2, space="PSUM")
            )
            agg_pool = ctx.enter_context(
                tc.tile_pool(name="agg", bufs=2, space="PSUM")
            )
            mini_ps = ctx.enter_context(
                tc.tile_pool(name="minips", bufs=2, space="PSUM")
            )

            # --- constants in SBUF ---
            ident_bf = const_pool.tile([P, P], BF, tag="identbf")
            make_identity(nc, ident_bf[:])
            ident_f = const_pool.tile([P, P], FP, tag="identf")
            make_identity(nc, ident_f[:])
            iota_f = const_pool.tile([P, P], BF, tag="iota")
            iota_i = const_pool.tile([P, P], mybir.dt.int32, tag="iotai")
            nc.gpsimd.iota(iota_i[:], pattern=[[1, P]], base=0,
                           channel_multiplier=0)
            nc.vector.tensor_copy(iota_f[:], iota_i[:])
            iotap_f = const_pool.tile([P, 1], FP, tag="iotap")
            iotap_i = const_pool.tile([P, 1], mybir.dt.int32, tag="iotapi")
            nc.gpsimd.iota(iotap_i[:], pattern=[[0, 1]], base=0,
                           channel_multiplier=1)
            nc.vector.tensor_copy(iotap_f[:], iotap_i[:])

            def load_const(h, shape, dt=BF, tag=None):
                t = const_pool.tile(shape, dt, tag=tag or h.name)
                nc.sync.dma_start(t[:], h[:])
                return t

            Wgc_sb = [load_const(w, [P, P]) for w in Wgc]
            Ws_sb = [load_const(w, [P, P]) for w in Ws]
            Wd_sb = [load_const(w, [P, P]) for w in Wd]
            arep_sb = [load_const(w, [P, P]) for w in arep]
            dstloc_sb = load_const(dstloc, [P, NW * T])
            ndw_sb = load_const(ndw, [P, NW], FP)
            ndnsw_sb = load_const(ndnsw, [P, NW], FP)
            idxlo_sb = load_const(idx_lo, [P, NW * L * P // 16], mybir.dt.int16)
            idxhi_sb = load_const(idx_hi, [P, NW * H * P // 16], mybir.dt.int16)

            def s_en_build_window(w):
                """Build S_en for all T tiles of window w: [128, T, 128]
                ([e, tile, n]) on the vector engine."""
                senw = sb_pool.tile([P, T, P], F8, tag="senw", bufs=2)
                nc.vector.tensor_tensor(
                    out=senw[:],
                    in0=dstloc_sb[:, w * T : (w + 1) * T]
                    .unsqueeze(2)
                    .to_broadcast([P, T, P]),
                    in1=iota_f[:].unsqueeze(1).to_broadcast([P, T, P]),
                    op=AO.is_equal,
                )
                return senw

            SLOAD_W = 2  # windows per S_en reload DMA

            def sden_prep():
                for w in range(NW):
                    senw = s_en_build_window(w)
                    nc.sync.dma_start(
                        sden[w].rearrange("p f -> p f"),
                        senw[:].rearrange("p t f -> p (t f)"),
                    )

            def s_en_load(w0, nwin):
                sload = sb_pool.tile([P, SLOAD_W, T * P], F8, tag="sload",
                                     bufs=2)
                nc.sync.dma_start(
                    sload[:, :nwin, :],
                    sden[w0 : w0 + nwin].rearrange("w p f -> p w f"),
                )
                return sload

            def snt4_build(w, n4):
                """Build transposed one-hots [node, e] for windows w..w+n4."""
                dst_rep4 = sb_pool.tile(
                    [P, 4, T * P], BF, tag="dstrep4", bufs=1, name="dst_rep4"
                )
                nc.sync.dma_start(
                    dst_rep4[:, :n4, :],
                    dstflat[w : w + n4, :]
                    .unsqueeze(0)
                    .to_broadcast([P, n4, T * P]),
                )
                snT4 = sb_pool.tile([P, 4, T, P], BF, tag="snT4", bufs=1,
                                    name="snT4")
                nc.vector.tensor_scalar(
                    out=snT4[:, :n4, :, :].rearrange("p w t f -> p (w t f)"),
                    in0=dst_rep4[:, :n4, :].rearrange("p w f -> p (w f)"),
                    scalar1=iotap_f[:, 0:1],
                    scalar2=None,
                    op0=AO.is_equal,
                )
                return snT4

            def gather_span(table_l, w0, nw, transpose):
                """Gather all edges of windows [w0, w0+nw). Returns
                (stg_lo, stg_hi): transpose -> [128, 1, n] column tiles,
                else [128, ntiles, 128] row tiles."""
                nlo, nhi = nw * L * P, nw * H * P
                outs = []
                for which, n, idx_sb, colpos in (
                    ("lo", nlo, idxlo_sb, w0 * L * P),
                    ("hi", nhi, idxhi_sb, w0 * H * P),
                ):
                    half = table_l[0:HALF, :] if which == "lo" else table_l[HALF:NP, :]
                    if transpose:
                        t = stg_pool.tile([P, 1, n], BF, tag=f"stg{which}")
                        o = t[:, :, :]
                    else:
                        t = stg_pool.tile([P, n // P, P], BF, tag=f"stg{which}")
                        o = t[:, :, :]
                    nc.gpsimd.dma_gather(
                        o,
                        half,
                        idx_sb[:, colpos // 16 : (colpos + n) // 16],
                        n,
                        n,
                        P,
                        transpose=transpose,
                        single_packet=False,
                    )
                    outs.append(t)
                return outs

            # =========================================================
            # GraphConv layers
            # =========================================================
            def gc_layer(li, table_l, W_sb, agin_out, scale_sb, htag):
                hnew = chunk_pool.tile([P, NW, P], BF, tag=htag)
                for (w0, nw) in spans:
                    stg_lo, stg_hi = gather_span(table_l, w0, nw, False)
                    for wr in range(nw):
                        w = w0 + wr
                        senw = s_en_build_window(w)
                        aggT_full = agg_pool.tile([P, P + 16], FP, tag="agg",
                                                  name="aggT")
                        aggT = aggT_full[:, :P]
                        for t in range(T):
                            if t < L:
                                lhs = stg_lo[:, wr * L + t, :]
                            else:
                                lhs = stg_hi[:, wr * H + (t - L), :]
                            nc.tensor.matmul(
                                out=aggT[:],
                                lhsT=lhs,
                                rhs=senw[:, t, :],
                                start=(t == 0),
                                stop=(t == T - 1),
                            )
                        aggT_sb = sb_pool.tile([P, P], BF, tag="aggTsb")
                        nc.scalar.copy(aggT_sb[:], aggT[:])
                        op = mini_ps.tile([P, P], FP, tag="mini")
                        nc.tensor.matmul(out=op[:], lhsT=aggT_sb[:], rhs=W_sb[:],
                                         start=True, stop=True)
                        nc.scalar.activation(
                            hnew[:, w, :], op[:], AFT.Relu,
                            scale=scale_sb[:, w : w + 1],
                        )
                nc.sync.dma_start(
                    agin_out[:].rearrange("(w p) f -> p w f", p=P), hnew[:]
                )
                return hnew

            # =========================================================
            # GATv2 layers
            # =========================================================
            def fdw_prep(h_tile, Wd_l, tag):
                """fd = h @ Wd per window, from the local chunk tile."""
                fdw = chunk_pool.tile([P, NW, P], BF, tag=tag)
                for w in range(NW):
                    tp = mini_ps.tile([P, P], BF, tag="mini")
                    nc.tensor.transpose(tp[:], h_tile[:, w, :], ident_bf[:])
                    hwT = sb_pool.tile([P, P], BF, tag="hwTsb")
                    nc.scalar.copy(hwT[:], tp[:])
                    fp = mini_ps.tile([P, P], FP, tag="mini")
                    nc.tensor.matmul(out=fp[:], lhsT=hwT[:], rhs=Wd_l[:],
                                     start=True, stop=True)
                    nc.scalar.copy(fdw[:, w, :], fp[:])
                return fdw

            def gat_layer(li, table_l, hprev, fdw, Ws_l, arep_l, agin_out,
                          htag, pool_state=None):
                hnew = chunk_pool.tile([P, NW, P], BF, tag=htag)
                for (w0, nw) in spans:
                    stg_lo, stg_hi = gather_span(table_l, w0, nw, True)
                    for wr in range(nw):
                        w = w0 + wr
                        if wr % 4 == 0:
                            snT4 = snt4_build(w, min(4, nw - wr))
                        if wr % SLOAD_W == 0:
                            sload = s_en_load(w, min(SLOAD_W, nw - wr))
                        snTw = snT4[:, wr % 4]  # [P, T, P]
                        swi = wr % SLOAD_W
                        agg = agg_pool.tile([P, P + 16], FP, tag="agg")
                        for g0 in range(0, T, GT):
                            gn = min(GT, T - g0)
                            eps = ps_pool.tile([P, GT * P], FP, tag="eps")
                            for k in range(gn):
                                t = g0 + k
                                if t < L:
                                    col = (wr * L + t) * P
                                    hsT = stg_lo[:, 0, col : col + P]
                                else:
                                    col = (wr * H + (t - L)) * P
                                    hsT = stg_hi[:, 0, col : col + P]
                                sl = slice(k * P, (k + 1) * P)
                                nc.tensor.matmul(out=eps[:, sl], lhsT=hsT,
                                                 rhs=Ws_l[:], start=True,
                                                 stop=False)
                                nc.tensor.matmul(out=eps[:, sl],
                                                 lhsT=snTw[:, t, :],
                                                 rhs=fdw[:, w, :], start=False,
                                                 stop=True)
                            epsv = eps[:, : gn * P].rearrange(
                                "p (a b) -> p a b", b=P
                            )
                            elr = sb_pool.tile([P, GT, P], BF, tag="elr",
                                               bufs=2)
                            nc.scalar.activation(elr[:, :gn, :], epsv,
                                                 AFT.Prelu, alpha=0.2)
                            prod = sb_pool.tile([P, GT, P], BF, tag="prod",
                                                bufs=2)
                            nc.vector.tensor_tensor(
                                out=prod[:, :gn, :], in0=elr[:, :gn, :],
                                in1=arep_l[:].unsqueeze(1).to_broadcast(
                                    [P, gn, P]
                                ),
                                op=AO.mult,
                            )
                            pv = prod[:, :gn, :].rearrange(
                                "p a (h d) -> p (a h) d", d=DH
                            )
                            fold8 = sb_pool.tile([P, GT * HEADS, 8], BF,
                                                 tag="fold8", bufs=2)
                            nc.vector.tensor_tensor(
                                out=fold8[:, : gn * HEADS, :],
                                in0=pv[:, :, 0:8], in1=pv[:, :, 8:16],
                                op=AO.add,
                            )
                            logit = sb_pool.tile([P, GT * HEADS], FP,
                                                 tag="logit")
                            nc.vector.tensor_reduce(
                                out=logit[:, : gn * HEADS],
                                in_=fold8[:, : gn * HEADS, :],
                                axis=mybir.AxisListType.X,
                                op=AO.add,
                            )
                            wf = sb_pool.tile([P, GT, P + 8], BF, tag="wf",
                                              bufs=2)
                            nc.scalar.activation(
                                wf[:, :gn, P : P + 8],
                                logit[:, : gn * HEADS].rearrange(
                                    "p (a b) -> p a b", b=HEADS
                                ),
                                AFT.Exp,
                            )
                            nc.vector.tensor_tensor(
                                out=wf[:, :gn, 0:P].rearrange(
                                    "p a (h d) -> p a h d", d=DH
                                ),
                                in0=eps[:, : gn * P].rearrange(
                                    "p (a h d) -> p a h d", h=HEADS, d=DH
                                ),
                                in1=wf[:, :gn, P : P + 8]
                                .unsqueeze(3)
                                .to_broadcast([P, gn, HEADS, DH]),
                                op=AO.mult,
                            )
                            for k in range(gn):
                                t = g0 + k
                                nc.tensor.matmul(
                                    out=agg[:, : P + 8],
                                    lhsT=sload[:, swi, t * P : (t + 1) * P],
                                    rhs=wf[:, k, :],
                                    start=(t == 0),
                                    stop=(t == T - 1),
                                )
                        # ---- window flush ----
                        sguard = sb_pool.tile([P, 8], FP, tag="sguard")
                        nc.vector.tensor_scalar_max(
                            sguard[:], agg[:, P : P + 8], 1e-30
                        )
                        rec = sb_pool.tile([P, 8], FP, tag="rec")
                        nc.vector.reciprocal(rec[:], sguard[:])
                        ind = sb_pool.tile([P, 8], BF, tag="ind")
                        nc.vector.tensor_scalar(
                            out=ind[:], in0=agg[:, P : P + 8],
                            scalar1=1e-20, scalar2=None, op0=AO.is_gt,
                        )
                        fdind = sb_pool.tile([P, P], BF, tag="fdind")
                        nc.gpsimd.tensor_tensor(
                            out=fdind[:].rearrange("p (h d) -> p h d", d=DH),
                            in0=fdw[:, w, :].rearrange("p (h d) -> p h d", d=DH),
                            in1=ind[:].unsqueeze(2).to_broadcast([P, HEADS, DH]),
                            op=AO.mult,
                        )
                        hmfd = sb_pool.tile([P, P], BF, tag="hmfd")
                        nc.gpsimd.tensor_tensor(
                            out=hmfd[:], in0=hprev[:, w, :], in1=fdind[:],
                            op=AO.subtract,
                        )
                        o2 = sb_pool.tile([P, P], FP, tag="o2")
                        nc.vector.tensor_tensor(
                            out=o2[:].rearrange("p (h d) -> p h d", d=DH),
                            in0=agg[:, 0:P].rearrange("p (h d) -> p h d", d=DH),
                            in1=rec[:].unsqueeze(2).to_broadcast([P, HEADS, DH]),
                            op=AO.mult,
                        )
                        o3 = sb_pool.tile([P, P], FP, tag="o3")
                        nc.gpsimd.tensor_tensor(
                            out=o3[:], in0=o2[:], in1=hmfd[:], op=AO.add
                        )
                        nc.scalar.activation(hnew[:, w, :], o3[:], AFT.Relu)
                        if pool_state is not None:
                            pool_window(pool_state, hnew, w)
                if agin_out is not None:
                    nc.sync.dma_start(
                        agin_out[:].rearrange("(w p) f -> p w f", p=P), hnew[:]
                    )
                return hnew

            # =========================================================
            # pooling (interleaved into the last GAT layer)
            # =========================================================
            def pool_window(st, hnew, w):
                tp = mini_ps.tile([P, P], BF, tag="mini")
                nc.tensor.transpose(tp[:], hnew[:, w, :], ident_bf[:])
                h5t = sb_pool.tile([P, P], BF, tag="h5t")
                nc.vector.tensor_copy(h5t[:], tp[:])
                if w % 8 == 0:
                    nw8 = min(8, NW - w)
                    st["pmask"] = sb_pool.tile(
                        [P, 8, KSEG * P], BF, tag="pmask8", bufs=1,
                        name="pmask_rep8"
                    )
                    nc.sync.dma_start(
                        st["pmask"][:, :nw8, :],
                        poolmask[w : w + nw8, :]
                        .unsqueeze(0)
                        .to_broadcast([P, nw8, KSEG * P]),
                    )
                msk = sb_pool.tile([P, KSEG, P], BF, tag="msk")
                nc.vector.tensor_tensor(
                    out=msk[:],
                    in0=h5t[:].unsqueeze(1).to_broadcast([P, KSEG, P]),
                    in1=st["pmask"][:, w % 8].rearrange(
                        "p (k b) -> p k b", b=P
                    ),
                    op=AO.min,
                )
                nc.vector.tensor_reduce(
                    out=st["stag"][:, w * KSEG : (w + 1) * KSEG],
                    in_=msk[:],
                    axis=mybir.AxisListType.X,
                    op=AO.max,
                )

            # =========================================================
            # forward pass
            # =========================================================
            h1 = gc_layer(0, tables[0], Wgc_sb[0], agin[0], ndnsw_sb, "hA")
            nc.gpsimd.collective_compute(
                "AllGather", AO.bypass, replica_groups=RG,
                ins=[agin[0].ap().opt()], outs=[tables[1].ap().opt()],
            )
            sden_prep()
            h2 = gc_layer(1, tables[1], Wgc_sb[1], agin[1], ndw_sb, "hB")
            fdw0 = fdw_prep(h2, Wd_sb[0], "fdwA")
            nc.gpsimd.collective_compute(
                "AllGather", AO.bypass, replica_groups=RG,
                ins=[agin[1].ap().opt()], outs=[tables[2].ap().opt()],
            )
            h3 = gat_layer(0, tables[2], h2, fdw0, Ws_sb[0], arep_sb[0],
                           agin[2], "hA")
            fdw1 = fdw_prep(h3, Wd_sb[1], "fdwB")
            nc.gpsimd.collective_compute(
                "AllGather", AO.bypass, replica_groups=RG,
                ins=[agin[2].ap().opt()], outs=[tables[3].ap().opt()],
            )
            h4 = gat_layer(1, tables[3], h3, fdw1, Ws_sb[1], arep_sb[1],
                           agin[3], "hB")
            fdw2 = fdw_prep(h4, Wd_sb[2], "fdwA")
            nc.gpsimd.collective_compute(
                "AllGather", AO.bypass, replica_groups=RG,
                ins=[agin[3].ap().opt()], outs=[tables[4].ap().opt()],
            )
            NSEG = NW * KSEG
            stag_t = chunk_pool.tile([P, NSEG], FP, tag="stag")
            pool_state = dict(stag=stag_t, pmask=None)
            gat_layer(2, tables[4], h4, fdw2, Ws_sb[2], arep_sb[2],
                      None, "hA", pool_state=pool_state)

            # =========================================================
            # graph-level max + MLP (replicated)
            # =========================================================
            stag = pool_state["stag"]
            gmask_all = sb_pool.tile([P, G, NSEG], BF, tag="gmaskall", bufs=1)
            nc.sync.dma_start(
                gmask_all[:],
                gmask[:].unsqueeze(0).to_broadcast([P, G, NSEG]),
            )
            gm = sb_pool.tile([P, G, NSEG], BF, tag="gm", bufs=1)
            nc.vector.tensor_tensor(
                out=gm[:],
                in0=stag[:, :NSEG].unsqueeze(1).to_broadcast([P, G, NSEG]),
                in1=gmask_all[:],
                op=AO.min,
            )
            hgT_part = sb_pool.tile([P, G], FP, tag="hgT_part")
            nc.vector.tensor_reduce(
                out=hgT_part[:], in_=gm[:],
                axis=mybir.AxisListType.X, op=AO.max,
            )
            nc.sync.dma_start(hgpart[:], hgT_part[:])
            nc.gpsimd.collective_compute(
                "AllGather", AO.bypass, replica_groups=RG,
                ins=[hgpart.ap().opt()], outs=[hgall.ap().opt()],
            )
            # final max over ranks: hgall rows = (r p)
            hgl = sb_pool.tile([P, N_CORES * G], FP, tag="hgl")
            nc.sync.dma_start(
                hgl[:].rearrange("p (r g) -> p r g", g=G),
                hgall[:].rearrange("(r p) g -> p r g", p=P),
            )
            hgT = sb_pool.tile([P, G], FP, tag="hgT")
            nc.vector.tensor_reduce(
                out=hgT[:],
                in_=hgl[:].rearrange("p (r g) -> p g r", g=G),
                axis=mybir.AxisListType.X, op=AO.max,
            )

            Wc1_sb = load_const(Wc1, [P, P], FP)
            Wc2_sb = load_const(Wc2, [P, 64], FP)
            Wc3_sb = load_const(Wc3, [64, OUT], FP)

            z1p = mini_ps.tile([G, P], FP, tag="mini")
            nc.tensor.matmul(out=z1p[:], lhsT=hgT[:], rhs=Wc1_sb[:],
                             start=True, stop=True)
            z1 = sb_pool.tile([G, P], FP, tag="z1")
            nc.scalar.activation(z1[:], z1p[:], AFT.Relu)
            z1Tp = mini_ps.tile([P, G], FP, tag="mini")
            nc.tensor.transpose(z1Tp[:], z1[:], ident_f[:G, :G])
            z1T = sb_pool.tile([P, G], FP, tag="z1T")
            nc.scalar.copy(z1T[:], z1Tp[:])
            z2p = mini_ps.tile([G, 64], FP, tag="mini")
            nc.tensor.matmul(out=z2p[:], lhsT=z1T[:], rhs=Wc2_sb[:],
                             start=True, stop=True)
            z2 = sb_pool.tile([G, 64], FP, tag="z2")
            nc.scalar.activation(z2[:], z2p[:], AFT.Relu)
            z2Tp = mini_ps.tile([64, G], FP, tag="mini")
            nc.tensor.transpose(z2Tp[:], z2[:], ident_f[:G, :G])
            z2T = sb_pool.tile([64, G], FP, tag="z2T")
            nc.scalar.copy(z2T[:], z2Tp[:])
            z3p = mini_ps.tile([G, OUT], FP, tag="mini")
            nc.tensor.matmul(out=z3p[:], lhsT=z2T[:], rhs=Wc3_sb[:],
                             start=True, stop=True)
            z3 = sb_pool.tile([G, OUT], FP, tag="z3")
            nc.scalar.copy(z3[:], z3p[:])
            nc.sync.dma_start(out_ext[:], z3[:])

    nc.compile()
    return nc


# ---------------------------------------------------------------------------
# Entry point
# ---------------------------------------------------------------------------

def _run(inputs, nw_per_core=49, trace=False):
    from concourse.bass_utils import run_bass_kernel_spmd

    src = np.asarray(inputs["src"])
    dst = np.asarray(inputs["dst"])
    n2g = np.asarray(inputs["node2graph"])
    feat = np.asarray(inputs["feature"], np.float32)

    cfg, per_core, ns, nd = prep(src, dst, n2g, nw_per_core)
    NP = cfg["NP"]

    featp = np.zeros((NP, P), np.float32)
    featp[: feat.shape[0]] = feat
    featp *= ns[:, None]
    table0 = featp.astype(bf16)

    def b(x):
        return np.ascontiguousarray(np.asarray(x, np.float32).astype(bf16))

    common = dict(
        table0=table0,
        Wgc0=b(inputs["W_gc1"]), Wgc1=b(inputs["W_gc2"]),
        Wc1=np.ascontiguousarray(np.asarray(inputs["Wc1"], np.float32)),
        Wc2=np.ascontiguousarray(np.asarray(inputs["Wc2"], np.float32)),
        Wc3=np.ascontiguousarray(np.asarray(inputs["Wc3"], np.float32)),
    )
    attn = np.asarray(inputs["attn"], np.float32)
    for i in range(3):
        common[f"Ws{i}"] = b(np.asarray(inputs["W_src"], np.float32)[i])
        common[f"Wd{i}"] = b(np.asarray(inputs["W_dst"], np.float32)[i])
        ar = np.broadcast_to(attn[i].reshape(1, HID), (P, HID))
        common[f"arep{i}"] = np.ascontiguousarray(ar).astype(bf16)

    in_maps = []
    for c in range(N_CORES):
        m = dict(common)
        m.update(per_core[c])
        in_maps.append(m)

    nc = build_nc(cfg)
    res = run_bass_kernel_spmd(nc, in_maps, core_ids=list(range(N_CORES)),
                               trace=trace)
    return np.asarray(res.results[0]["out"], np.float32), res


def kernel(**inputs) -> np.ndarray:
    out, _ = _run(inputs)
    return out


# revision 10
# speedup vs baseline: 1.1318x; 1.0752x over previous
"""Trainium2 Bass kernel for nn_DifferentPooling (GNN message passing).

Strategy (8 NeuronCores, SPMD):
  - Nodes padded to NP = 8*CHUNK and partitioned by node id across cores.
  - Edges partitioned by dst core; within a core, grouped into 128-node dst
    "windows". Aggregation (segment sum / softmax-sum) is done per window via
    one-hot selection matrices multiplied on the TensorEngine, accumulating in
    PSUM across the window's 128-edge tiles.
  - Feature rows are fetched with batched dma_gather (int16 indices, so the
    node table is split into two halves and each window's edges are split
    into lo/hi groups, each padded to 128-edge tiles).
  - After each layer, per-core node-feature chunks are AllGather'd so every
    core has the full table for the next layer's gathers.
  - GATv2 softmax uses exp(logit) without max subtraction (logits here are
    tiny), alpha = p / segsum(p) with a 1e-30 guard. The weighted aggregation
    uses sum_e w*eps = sum_e w*(fs+fd); since sum alpha = 1 per dst node,
    out = agg_eps/s - fd (fd subtracted only where the node has edges).
  - One-hot selection matrices (S_en and its transpose) are rebuilt on the
    Vector engine per window instead of being cached in DRAM.
  - Graph max-pooling: per-window masked-max segments interleaved into the
    last GAT layer -> small AllGather -> final max and a replicated fp32 MLP.

All biases in this problem are zeros by spec (fill="zeros"); they are not
applied on device.
"""

import sys

sys.path.insert(0, "/opt/trn_rl_repo")

import numpy as np
import ml_dtypes

bf16 = ml_dtypes.bfloat16

N_CORES = 8
P = 128  # window size / partition count
N_REAL = 50000
E_REAL = 500000
G = 64
HID = 128
HEADS = 8
DH = 16
OUT = 256


# ---------------------------------------------------------------------------
# Host-side preprocessing
# ---------------------------------------------------------------------------

def _wrap_idx(arr):
    """int idx array (len % 16 == 0) -> [128, len/16] int16 wrapped layout:
    idx i lives at [i % 16, i // 16], replicated across the 8 groups of 16
    partitions (one per Q7 core)."""
    a = np.asarray(arr, np.int16).reshape(-1, 16).T  # [16, cols]
    return np.tile(a, (8, 1))  # [128, cols]


def prep(src, dst, node2graph, nw_per_core):
    """Build per-core edge/window metadata. Returns (cfg, host arrays)."""
    NW = nw_per_core
    CHUNK = NW * P
    NP = N_CORES * CHUNK
    HALF = NP // 2
    N = len(node2graph)
    E = len(src)

    src = np.asarray(src, np.int64)
    dst = np.asarray(dst, np.int64)
    n2g = np.asarray(node2graph, np.int64)

    outdeg = np.zeros(NP, np.float32)
    np.add.at(outdeg, src, 1.0)
    indeg = np.zeros(NP, np.float32)
    np.add.at(indeg, dst, 1.0)
    ns = np.maximum(outdeg, 1.0) ** -0.5
    nd = np.maximum(indeg, 1.0) ** -0.5

    # sort edges by dst, bucket into windows
    order = np.argsort(dst, kind="stable")
    sdst = dst[order]
    ssrc = src[order]
    n_win_total = NP // P
    win_starts = np.searchsorted(sdst, np.arange(0, NP + 1, P))

    # per (global window): lo/hi edge lists sorted by src
    lo_lists, hi_lists = [], []
    max_lo = max_hi = 1
    for w in range(n_win_total):
        a, b = win_starts[w], win_starts[w + 1]
        es, ed = ssrc[a:b], sdst[a:b] - w * P
        m = es < HALF
        ordl = np.argsort(es[m], kind="stable")
        ordh = np.argsort(es[~m], kind="stable")
        lo_lists.append((es[m][ordl], ed[m][ordl]))
        hi_lists.append((es[~m][ordh] - HALF, ed[~m][ordh]))
        max_lo = max(max_lo, len(lo_lists[-1][0]))
        max_hi = max(max_hi, len(hi_lists[-1][0]))

    L = (max_lo + P - 1) // P
    H = (max_hi + P - 1) // P
    T = L + H

    # spans of SPAN_W windows (gather batching granularity)
    SPAN_W = 6 if NW >= 6 else 2
    spans = []
    w0 = 0
    while w0 < NW:
        spans.append((w0, min(SPAN_W, NW - w0)))
        w0 += SPAN_W

    per_core = []
    for c in range(N_CORES):
        idx_lo = np.zeros((NW, L * P), np.int64)
        dst_lo = np.full((NW, L * P), P, np.int64)  # sentinel 128
        idx_hi = np.zeros((NW, H * P), np.int64)
        dst_hi = np.full((NW, H * P), P, np.int64)
        for w in range(NW):
            el, dl = lo_lists[c * NW + w]
            eh, dh_ = hi_lists[c * NW + w]
            idx_lo[w, : len(el)] = el
            dst_lo[w, : len(dl)] = dl
            idx_hi[w, : len(eh)] = eh
            dst_hi[w, : len(dh_)] = dh_
        # dstloc: [NW*T, 128] -> transpose to [128, NW*T]; col w*T+t
        dstloc = np.concatenate(
            [dst_lo.reshape(NW, L, P), dst_hi.reshape(NW, H, P)], axis=1
        ).reshape(NW * T, P)
        ndw = nd[c * CHUNK : (c + 1) * CHUNK].reshape(NW, P).T.copy()
        nsw = ns[c * CHUNK : (c + 1) * CHUNK].reshape(NW, P).T.copy()
        per_core.append(
            dict(
                idx_lo=_wrap_idx(idx_lo.reshape(-1)),
                idx_hi=_wrap_idx(idx_hi.reshape(-1)),
                dstloc=np.ascontiguousarray(dstloc.T).astype(bf16),
                dstflat=np.ascontiguousarray(
                    dstloc.reshape(NW, T * P)).astype(bf16),
                ndw=np.ascontiguousarray(ndw, np.float32),
                ndnsw=np.ascontiguousarray(ndw * nsw, np.float32),
            )
        )

    # pooling segments per core: runs of equal graph id inside each window
    n2g_pad = np.full(NP, -1, np.int64)
    n2g_pad[:N] = n2g
    seg_all = []  # per core: list of (w, j0, j1, g)
    KSEG = 1
    for c in range(N_CORES):
        segs = []
        for w in range(NW):
            ids = n2g_pad[c * CHUNK + w * P : c * CHUNK + (w + 1) * P]
            j = 0
            wsegs = []
            while j < P:
                g = ids[j]
                k = j
                while k < P and ids[k] == g:
                    k += 1
                if g >= 0:
                    wsegs.append((j, k, int(g)))
                j = k
            KSEG = max(KSEG, len(wsegs))
            segs.append(wsegs)
        seg_all.append(segs)

    BIG = np.float32(1e30)
    NSEG = NW * KSEG
    for c in range(N_CORES):
        maskvec = np.full((NW, KSEG, P), -BIG, np.float32)
        gmask = np.full((G, NSEG), -BIG, np.float32)
        for w in range(NW):
            for k, (j0, j1, g) in enumerate(seg_all[c][w]):
                maskvec[w, k, j0:j1] = BIG
                gmask[g, w * KSEG + k] = BIG
        per_core[c]["poolmask"] = maskvec.reshape(NW, KSEG * P).astype(bf16)
        per_core[c]["gmask"] = gmask.astype(bf16)

    cfg = dict(NW=NW, CHUNK=CHUNK, NP=NP, HALF=HALF, L=L, H=H, T=T,
               spans=spans, KSEG=KSEG)
    return cfg, per_core, ns, nd


# ---------------------------------------------------------------------------
# Bass kernel builder
# ---------------------------------------------------------------------------

def build_nc(cfg):
    import concourse.bacc as bacc
    import concourse.bass as bass
    import concourse.mybir as mybir
    import concourse.tile as tile
    from concourse.masks import make_identity

    NW, CHUNK, NP, HALF = cfg["NW"], cfg["CHUNK"], cfg["NP"], cfg["HALF"]
    L, H, T, spans, KSEG = cfg["L"], cfg["H"], cfg["T"], cfg["spans"], cfg["KSEG"]
    FP = mybir.dt.float32
    BF = mybir.dt.bfloat16
    F8 = mybir.dt.float8e4
    AO = mybir.AluOpType
    AFT = mybir.ActivationFunctionType
    GT = 6  # edge tiles per PSUM group in GAT layers

    nc = bacc.Bacc("TRN2", target_bir_lowering=False, debug=False,
                   num_devices=N_CORES)

    def din(name, shape, dt=BF):
        return nc.dram_tensor(name, shape, dt, kind="ExternalInput")

    table0 = din("table0", [NP, P])
    Wgc = [din(f"Wgc{i}", [P, P]) for i in range(2)]
    Ws = [din(f"Ws{i}", [P, P]) for i in range(3)]
    Wd = [din(f"Wd{i}", [P, P]) for i in range(3)]
    arep = [din(f"arep{i}", [P, P]) for i in range(3)]
    Wc1 = din("Wc1", [P, P], FP)
    Wc2 = din("Wc2", [P, 64], FP)
    Wc3 = din("Wc3", [64, OUT], FP)
    idx_lo = din("idx_lo", [P, NW * L * P // 16], mybir.dt.int16)
    idx_hi = din("idx_hi", [P, NW * H * P // 16], mybir.dt.int16)
    dstloc = din("dstloc", [P, NW * T])
    dstflat = din("dstflat", [NW, T * P])
    ndw = din("ndw", [P, NW], FP)
    ndnsw = din("ndnsw", [P, NW], FP)
    poolmask = din("poolmask", [NW, KSEG * P])
    gmask = din("gmask", [G, NW * KSEG])

    out_ext = nc.dram_tensor("out", [G, OUT], FP, kind="ExternalOutput")

    # internal DRAM
    agin = [nc.dram_tensor(f"agin{i}", [CHUNK, P], BF) for i in range(4)]
    tables = [table0] + [
        nc.dram_tensor(f"table{i+1}", [NP, P], BF, addr_space="Shared")
        for i in range(4)
    ]
    sden = nc.dram_tensor("sden", [NW, P, T * P], mybir.dt.float8e4)
    hgpart = nc.dram_tensor("hgpart", [P, G], FP)
    hgall = nc.dram_tensor("hgall", [N_CORES * P, G], FP, addr_space="Shared")

    RG = [list(range(N_CORES))]

    with tile.TileContext(nc) as tc:
        import contextlib

        ctx = contextlib.ExitStack()
        with ctx:
            const_pool = ctx.enter_context(tc.tile_pool(name="const", bufs=1))
            stg_pool = ctx.enter_context(tc.tile_pool(name="stg", bufs=2))
            sb_pool = ctx.enter_context(tc.tile_pool(name="sb", bufs=3))
            chunk_pool = ctx.enter_context(tc.tile_pool(name="chunk", bufs=1))
            ps_pool = ctx.enter_context(
                tc.tile_pool(name="ps", bufs=2, space="PSUM")
            )
            agg_pool = ctx.enter_context(
                tc.tile_pool(name="agg", bufs=2, space="PSUM")
            )
            mini_ps = ctx.enter_context(
                tc.tile_pool(name="minips", bufs=2, space="PSUM")
            )

            # --- constants in SBUF ---
            ident_bf = const_pool.tile([P, P], BF, tag="identbf")
            make_identity(nc, ident_bf[:])
            ident_f = const_pool.tile([P, P], FP, tag="identf")
            make_identity(nc, ident_f[:])
            iota_f = const_pool.tile([P, P], BF, tag="iota")
            iota_i = const_pool.tile([P, P], mybir.dt.int32, tag="iotai")
            nc.gpsimd.iota(iota_i[:], pattern=[[1, P]], base=0,
                           channel_multiplier=0)
            nc.vector.tensor_copy(iota_f[:], iota_i[:])
            iotap_f = const_pool.tile([P, 1], FP, tag="iotap")
            iotap_i = const_pool.tile([P, 1], mybir.dt.int32, tag="iotapi")
            nc.gpsimd.iota(iotap_i[:], pattern=[[0, 1]], base=0,
                           channel_multiplier=1)
            nc.vector.tensor_copy(iotap_f[:], iotap_i[:])

            def load_const(h, shape, dt=BF, tag=None):
                t = const_pool.tile(shape, dt, tag=tag or h.name)
                nc.sync.dma_start(t[:], h[:])
                return t

            Wgc_sb = [load_const(w, [P, P]) for w in Wgc]
            Ws_sb = [load_const(w, [P, P]) for w in Ws]
            Wd_sb = [load_const(w, [P, P]) for w in Wd]
            arep_sb = [load_const(w, [P, P]) for w in arep]
            dstloc_sb = load_const(dstloc, [P, NW * T])
            ndw_sb = load_const(ndw, [P, NW], FP)
            ndnsw_sb = load_const(ndnsw, [P, NW], FP)
            idxlo_sb = load_const(idx_lo, [P, NW * L * P // 16], mybir.dt.int16)
            idxhi_sb = load_const(idx_hi, [P, NW * H * P // 16], mybir.dt.int16)

            def s_en_build_window(w):
                """Build S_en for all T tiles of window w: [128, T, 128]
                ([e, tile, n]) on the vector engine."""
                senw = sb_pool.tile([P, T, P], F8, tag="senw", bufs=2)
                nc.vector.tensor_tensor(
                    out=senw[:],
                    in0=dstloc_sb[:, w * T : (w + 1) * T]
                    .unsqueeze(2)
                    .to_broadcast([P, T, P]),
                    in1=iota_f[:].unsqueeze(1).to_broadcast([P, T, P]),
                    op=AO.is_equal,
                )
                return senw

            SLOAD_W = 2  # windows per S_en reload DMA

            def sden_prep():
                for w in range(NW):
                    senw = s_en_build_window(w)
                    nc.sync.dma_start(
                        sden[w].rearrange("p f -> p f"),
                        senw[:].rearrange("p t f -> p (t f)"),
                    )

            def s_en_load(w0, nwin):
                sload = sb_pool.tile([P, SLOAD_W, T * P], F8, tag="sload",
                                     bufs=2)
                nc.sync.dma_start(
                    sload[:, :nwin, :],
                    sden[w0 : w0 + nwin].rearrange("w p f -> p w f"),
                )
                return sload

            def snt4_build(w, n4):
                """Build transposed one-hots [node, e] for windows w..w+n4."""
                dst_rep4 = sb_pool.tile(
                    [P, 4, T * P], BF, tag="dstrep4", bufs=1, name="dst_rep4"
                )
                nc.sync.dma_start(
                    dst_rep4[:, :n4, :],
                    dstflat[w : w + n4, :]
                    .unsqueeze(0)
                    .to_broadcast([P, n4, T * P]),
                )
                snT4 = sb_pool.tile([P, 4, T, P], BF, tag="snT4", bufs=1,
                                    name="snT4")
                nc.vector.tensor_scalar(
                    out=snT4[:, :n4, :, :].rearrange("p w t f -> p (w t f)"),
                    in0=dst_rep4[:, :n4, :].rearrange("p w f -> p (w f)"),
                    scalar1=iotap_f[:, 0:1],
                    scalar2=None,
                    op0=AO.is_equal,
                )
                return snT4

            def gather_span(table_l, w0, nw, transpose):
                """Gather all edges of windows [w0, w0+nw). Returns
                (stg_lo, stg_hi): transpose -> [128, 1, n] column tiles,
                else [128, ntiles, 128] row tiles."""
                nlo, nhi = nw * L * P, nw * H * P
                outs = []
                for which, n, idx_sb, colpos in (
                    ("lo", nlo, idxlo_sb, w0 * L * P),
                    ("hi", nhi, idxhi_sb, w0 * H * P),
                ):
                    half = table_l[0:HALF, :] if which == "lo" else table_l[HALF:NP, :]
                    if transpose:
                        t = stg_pool.tile([P, 1, n], BF, tag=f"stg{which}")
                        o = t[:, :, :]
                    else:
                        t = stg_pool.tile([P, n // P, P], BF, tag=f"stg{which}")
                        o = t[:, :, :]
                    nc.gpsimd.dma_gather(
                        o,
                        half,
                        idx_sb[:, colpos // 16 : (colpos + n) // 16],
                        n,
                        n,
                        P,
                        transpose=transpose,
                        single_packet=False,
                    )
                    outs.append(t)
                return outs

            # =========================================================
            # GraphConv layers
            # =========================================================
            def gc_layer(li, table_l, W_sb, agin_out, scale_sb, htag):
                hnew = chunk_pool.tile([P, NW, P], BF, tag=htag)
                for (w0, nw) in spans:
                    stg_lo, stg_hi = gather_span(table_l, w0, nw, False)
                    for wr in range(nw):
                        w = w0 + wr
                        senw = s_en_build_window(w)
                        aggT_full = agg_pool.tile([P, P + 16], FP, tag="agg",
                                                  name="aggT")
                        aggT = aggT_full[:, :P]
                        for t in range(T):
                            if t < L:
                                lhs = stg_lo[:, wr * L + t, :]
                            else:
                                lhs = stg_hi[:, wr * H + (t - L), :]
                            nc.tensor.matmul(
                                out=aggT[:],
                                lhsT=lhs,
                                rhs=senw[:, t, :],
                                start=(t == 0),
                                stop=(t == T - 1),
                            )
                        aggT_sb = sb_pool.tile([P, P], BF, tag="aggTsb")
                        nc.scalar.copy(aggT_sb[:], aggT[:])
                        op = mini_ps.tile([P, P], FP, tag="mini")
                        nc.tensor.matmul(out=op[:], lhsT=aggT_sb[:], rhs=W_sb[:],
                                         start=True, stop=True)
                        nc.scalar.activation(
                            hnew[:, w, :], op[:], AFT.Relu,
                            scale=scale_sb[:, w : w + 1],
                        )
                nc.sync.dma_start(
                    agin_out[:].rearrange("(w p) f -> p w f", p=P), hnew[:]
                )
                return hnew

            # =========================================================
            # GATv2 layers
            # =========================================================
            def fdw_prep(h_tile, Wd_l, tag):
                """fd = h @ Wd per window, from the local chunk tile."""
                fdw = chunk_pool.tile([P, NW, P], BF, tag=tag)
                for w in range(NW):
                    tp = mini_ps.tile([P, P], BF, tag="mini")
                    nc.tensor.transpose(tp[:], h_tile[:, w, :], ident_bf[:])
                    hwT = sb_pool.tile([P, P], BF, tag="hwTsb")
                    nc.scalar.copy(hwT[:], tp[:])
                    fp = mini_ps.tile([P, P], FP, tag="mini")
                    nc.tensor.matmul(out=fp[:], lhsT=hwT[:], rhs=Wd_l[:],
                                     start=True, stop=True)
                    nc.scalar.copy(fdw[:, w, :], fp[:])
                return fdw

            def gat_layer(li, table_l, hprev, fdw, Ws_l, arep_l, agin_out,
                          htag, pool_state=None):
                hnew = chunk_pool.tile([P, NW, P], BF, tag=htag)
                for (w0, nw) in spans:
                    stg_lo, stg_hi = gather_span(table_l, w0, nw, True)
                    for wr in range(nw):
                        w = w0 + wr
                        if wr % 4 == 0:
                            snT4 = snt4_build(w, min(4, nw - wr))
                        if wr % SLOAD_W == 0:
                            sload = s_en_load(w, min(SLOAD_W, nw - wr))
                        snTw = snT4[:, wr % 4]  # [P, T, P]
                        swi = wr % SLOAD_W
                        agg = agg_pool.tile([P, P + 16], FP, tag="agg")
                        for g0 in range(0, T, GT):
                            gn = min(GT, T - g0)
                            eps = ps_pool.tile([P, GT * P], FP, tag="eps")
                            for k in range(gn):
                                t = g0 + k
                                if t < L:
                                    col = (wr * L + t) * P
                                    hsT = stg_lo[:, 0, col : col + P]
                                else:
                                    col = (wr * H + (t - L)) * P
                                    hsT = stg_hi[:, 0, col : col + P]
                                sl = slice(k * P, (k + 1) * P)
                                nc.tensor.matmul(out=eps[:, sl], lhsT=hsT,
                                                 rhs=Ws_l[:], start=True,
                                                 stop=False)
                                nc.tensor.matmul(out=eps[:, sl],
                                                 lhsT=snTw[:, t, :],
                                                 rhs=fdw[:, w, :], start=False,
                                                 stop=True)
                            epsv = eps[:, : gn * P].rearrange(
                                "p (a b) -> p a b", b=P
                            )
                            elr = sb_pool.tile([P, GT, P], BF, tag="elr",
                                               bufs=2)
                            nc.scalar.activation(elr[:, :gn, :], epsv,
                                                 AFT.Prelu, alpha=0.2)
                            eps_sb = sb_pool.tile([P, GT, P], BF, tag="epssb",
                                                  bufs=2)
                            nc.scalar.copy(eps_sb[:, :gn, :], epsv)
                            prod = sb_pool.tile([P, GT, P], BF, tag="prod",
                                                bufs=2)
                            nc.vector.tensor_tensor(
                                out=prod[:, :gn, :], in0=elr[:, :gn, :],
                                in1=arep_l[:].unsqueeze(1).to_broadcast(
                                    [P, gn, P]
                                ),
                                op=AO.mult,
                            )
                            pv = prod[:, :gn, :].rearrange(
                                "p a (h d) -> p (a h) d", d=DH
                            )
                            fold8 = sb_pool.tile([P, GT * HEADS, 8], BF,
                                                 tag="fold8", bufs=2)
                            nc.vector.tensor_tensor(
                                out=fold8[:, : gn * HEADS, :],
                                in0=pv[:, :, 0:8], in1=pv[:, :, 8:16],
                                op=AO.add,
                            )
                            logit = sb_pool.tile([P, GT * HEADS], FP,
                                                 tag="logit")
                            nc.vector.tensor_reduce(
                                out=logit[:, : gn * HEADS],
                                in_=fold8[:, : gn * HEADS, :],
                                axis=mybir.AxisListType.X,
                                op=AO.add,
                            )
                            wf = sb_pool.tile([P, GT, P + 8], BF, tag="wf",
                                              bufs=2)
                            nc.scalar.activation(
                                wf[:, :gn, P : P + 8],
                                logit[:, : gn * HEADS].rearrange(
                                    "p (a b) -> p a b", b=HEADS
                                ),
                                AFT.Exp,
                            )
                            nc.vector.tensor_tensor(
                                out=wf[:, :gn, 0:P].rearrange(
                                    "p a (h d) -> p a h d", d=DH
                                ),
                                in0=eps_sb[:, :gn, :].rearrange(
                                    "p a (h d) -> p a h d", d=DH
                                ),
                                in1=wf[:, :gn, P : P + 8]
                                .unsqueeze(3)
                                .to_broadcast([P, gn, HEADS, DH]),
                                op=AO.mult,
                            )
                            for k in range(gn):
                                t = g0 + k
                                nc.tensor.matmul(
                                    out=agg[:, : P + 8],
                                    lhsT=sload[:, swi, t * P : (t + 1) * P],
                                    rhs=wf[:, k, :],
                                    start=(t == 0),
                                    stop=(t == T - 1),
                                )
                        # ---- window flush ----
                        sguard = sb_pool.tile([P, 8], FP, tag="sguard")
                        nc.vector.tensor_scalar_max(
                            sguard[:], agg[:, P : P + 8], 1e-30
                        )
                        rec = sb_pool.tile([P, 8], FP, tag="rec")
                        nc.vector.reciprocal(rec[:], sguard[:])
                        ind = sb_pool.tile([P, 8], BF, tag="ind")
                        nc.vector.tensor_scalar(
                            out=ind[:], in0=agg[:, P : P + 8],
                            scalar1=1e-20, scalar2=None, op0=AO.is_gt,
                        )
                        fdind = sb_pool.tile([P, P], BF, tag="fdind")
                        nc.gpsimd.tensor_tensor(
                            out=fdind[:].rearrange("p (h d) -> p h d", d=DH),
                            in0=fdw[:, w, :].rearrange("p (h d) -> p h d", d=DH),
                            in1=ind[:].unsqueeze(2).to_broadcast([P, HEADS, DH]),
                            op=AO.mult,
                        )
                        hmfd = sb_pool.tile([P, P], BF, tag="hmfd")
                        nc.gpsimd.tensor_tensor(
                            out=hmfd[:], in0=hprev[:, w, :], in1=fdind[:],
                            op=AO.subtract,
                        )
                        o2 = sb_pool.tile([P, P], FP, tag="o2")
                        nc.vector.tensor_tensor(
                            out=o2[:].rearrange("p (h d) -> p h d", d=DH),
                            in0=agg[:, 0:P].rearrange("p (h d) -> p h d", d=DH),
                            in1=rec[:].unsqueeze(2).to_broadcast([P, HEADS, DH]),
                            op=AO.mult,
                        )
                        o3 = sb_pool.tile([P, P], FP, tag="o3")
                        nc.gpsimd.tensor_tensor(
                            out=o3[:], in0=o2[:], in1=hmfd[:], op=AO.add
                        )
                        nc.scalar.activation(hnew[:, w, :], o3[:], AFT.Relu)
                        if pool_state is not None:
                            pool_window(pool_state, hnew, w)
                if agin_out is not None:
                    nc.sync.dma_start(
                        agin_out[:].rearrange("(w p) f -> p w f", p=P), hnew[:]
                    )
                return hnew

            # =========================================================
            # pooling (interleaved into the last GAT layer)
            # =========================================================
            def pool_window(st, hnew, w):
                tp = mini_ps.tile([P, P], BF, tag="mini")
                nc.tensor.transpose(tp[:], hnew[:, w, :], ident_bf[:])
                h5t = sb_pool.tile([P, P], BF, tag="h5t")
                nc.vector.tensor_copy(h5t[:], tp[:])
                if w % 8 == 0:
                    nw8 = min(8, NW - w)
                    st["pmask"] = sb_pool.tile(
                        [P, 8, KSEG * P], BF, tag="pmask8", bufs=1,
                        name="pmask_rep8"
                    )
                    nc.sync.dma_start(
                        st["pmask"][:, :nw8, :],
                        poolmask[w : w + nw8, :]
                        .unsqueeze(0)
                        .to_broadcast([P, nw8, KSEG * P]),
                    )
                msk = sb_pool.tile([P, KSEG, P], BF, tag="msk")
                nc.vector.tensor_tensor(
                    out=msk[:],
                    in0=h5t[:].unsqueeze(1).to_broadcast([P, KSEG, P]),
                    in1=st["pmask"][:, w % 8].rearrange(
                        "p (k b) -> p k b", b=P
                    ),
                    op=AO.min,
                )
                nc.vector.tensor_reduce(
                    out=st["stag"][:, w * KSEG : (w + 1) * KSEG],
                    in_=msk[:],
                    axis=mybir.AxisListType.X,
                    op=AO.max,
                )

            # =========================================================
            # forward pass
            # =========================================================
            h1 = gc_layer(0, tables[0], Wgc_sb[0], agin[0], ndnsw_sb, "hA")
            nc.gpsimd.collective_compute(
                "AllGather", AO.bypass, replica_groups=RG,
                ins=[agin[0].ap().opt()], outs=[tables[1].ap().opt()],
            )
            sden_prep()
            h2 = gc_layer(1, tables[1], Wgc_sb[1], agin[1], ndw_sb, "hB")
            fdw0 = fdw_prep(h2, Wd_sb[0], "fdwA")
            nc.gpsimd.collective_compute(
                "AllGather", AO.bypass, replica_groups=RG,
                ins=[agin[1].ap().opt()], outs=[tables[2].ap().opt()],
            )
            h3 = gat_layer(0, tables[2], h2, fdw0, Ws_sb[0], arep_sb[0],
                           agin[2], "hA")
            fdw1 = fdw_prep(h3, Wd_sb[1], "fdwB")
            nc.gpsimd.collective_compute(
                "AllGather", AO.bypass, replica_groups=RG,
                ins=[agin[2].ap().opt()], outs=[tables[3].ap().opt()],
            )
            h4 = gat_layer(1, tables[3], h3, fdw1, Ws_sb[1], arep_sb[1],
                           agin[3], "hB")
            fdw2 = fdw_prep(h4, Wd_sb[2], "fdwA")
            nc.gpsimd.collective_compute(
                "AllGather", AO.bypass, replica_groups=RG,
                ins=[agin[3].ap().opt()], outs=[tables[4].ap().opt()],
            )
            NSEG = NW * KSEG
            stag_t = chunk_pool.tile([P, NSEG], FP, tag="stag")
            pool_state = dict(stag=stag_t, pmask=None)
            gat_layer(2, tables[4], h4, fdw2, Ws_sb[2], arep_sb[2],
                      None, "hA", pool_state=pool_state)

            # =========================================================
            # graph-level max + MLP (replicated)
            # =========================================================
            stag = pool_state["stag"]
            gmask_all = sb_pool.tile([P, G, NSEG], BF, tag="gmaskall", bufs=1)
            nc.sync.dma_start(
                gmask_all[:],
                gmask[:].unsqueeze(0).to_broadcast([P, G, NSEG]),
            )
            gm = sb_pool.tile([P, G, NSEG], BF, tag="gm", bufs=1)
            nc.vector.tensor_tensor(
                out=gm[:],
                in0=stag[:, :NSEG].unsqueeze(1).to_broadcast([P, G, NSEG]),
                in1=gmask_all[:],
                op=AO.min,
            )
            hgT_part = sb_pool.tile([P, G], FP, tag="hgT_part")
            nc.vector.tensor_reduce(
                out=hgT_part[:], in_=gm[:],
                axis=mybir.AxisListType.X, op=AO.max,
            )
            nc.sync.dma_start(hgpart[:], hgT_part[:])
            nc.gpsimd.collective_compute(
                "AllGather", AO.bypass, replica_groups=RG,
                ins=[hgpart.ap().opt()], outs=[hgall.ap().opt()],
            )
            # final max over ranks: hgall rows = (r p)
            hgl = sb_pool.tile([P, N_CORES * G], FP, tag="hgl")
            nc.sync.dma_start(
                hgl[:].rearrange("p (r g) -> p r g", g=G),
                hgall[:].rearrange("(r p) g -> p r g", p=P),
            )
            hgT = sb_pool.tile([P, G], FP, tag="hgT")
            nc.vector.tensor_reduce(
                out=hgT[:],
                in_=hgl[:].rearrange("p (r g) -> p g r", g=G),
                axis=mybir.AxisListType.X, op=AO.max,
            )

            Wc1_sb = load_const(Wc1, [P, P], FP)
            Wc2_sb = load_const(Wc2, [P, 64], FP)
            Wc3_sb = load_const(Wc3, [64, OUT], FP)

            z1p = mini_ps.tile([G, P], FP, tag="mini")
            nc.tensor.matmul(out=z1p[:], lhsT=hgT[:], rhs=Wc1_sb[:],
                             start=True, stop=True)
            z1 = sb_pool.tile([G, P], FP, tag="z1")
            nc.scalar.activation(z1[:], z1p[:], AFT.Relu)
            z1Tp = mini_ps.tile([P, G], FP, tag="mini")
            nc.tensor.transpose(z1Tp[:], z1[:], ident_f[:G, :G])
            z1T = sb_pool.tile([P, G], FP, tag="z1T")
            nc.scalar.copy(z1T[:], z1Tp[:])
            z2p = mini_ps.tile([G, 64], FP, tag="mini")
            nc.tensor.matmul(out=z2p[:], lhsT=z1T[:], rhs=Wc2_sb[:],
                             start=True, stop=True)
            z2 = sb_pool.tile([G, 64], FP, tag="z2")
            nc.scalar.activation(z2[:], z2p[:], AFT.Relu)
            z2Tp = mini_ps.tile([64, G], FP, tag="mini")
            nc.tensor.transpose(z2Tp[:], z2[:], ident_f[:G, :G])
            z2T = sb_pool.tile([64, G], FP, tag="z2T")
            nc.scalar.copy(z2T[:], z2Tp[:])
            z3p = mini_ps.tile([G, OUT], FP, tag="mini")
            nc.tensor.matmul(out=z3p[:], lhsT=z2T[:], rhs=Wc3_sb[:],
                             start=True, stop=True)
            z3 = sb_pool.tile([G, OUT], FP, tag="z3")
            nc.scalar.copy(z3[:], z3p[:])
            nc.sync.dma_start(out_ext[:], z3[:])

    nc.compile()
    return nc


# ---------------------------------------------------------------------------
# Entry point
# ---------------------------------------------------------------------------

def _run(inputs, nw_per_core=49, trace=False):
    from concourse.bass_utils import run_bass_kernel_spmd

    src = np.asarray(inputs["src"])
    dst = np.asarray(inputs["dst"])
    n2g = np.asarray(inputs["node2graph"])
    feat = np.asarray(inputs["feature"], np.float32)

    cfg, per_core, ns, nd = prep(src, dst, n2g, nw_per_core)
    NP = cfg["NP"]

    featp = np.zeros((NP, P), np.float32)
    featp[: feat.shape[0]] = feat
    featp *= ns[:, None]
    table0 = featp.astype(bf16)

    def b(x):
        return np.ascontiguousarray(np.asarray(x, np.float32).astype(bf16))

    common = dict(
        table0=table0,
        Wgc0=b(inputs["W_gc1"]), Wgc1=b(inputs["W_gc2"]),
        Wc1=np.ascontiguousarray(np.asarray(inputs["Wc1"], np.float32)),
        Wc2=np.ascontiguousarray(np.asarray(inputs["Wc2"], np.float32)),
        Wc3=np.ascontiguousarray(np.asarray(inputs["Wc3"], np.float32)),
    )
    attn = np.asarray(inputs["attn"], np.float32)
    for i in range(3):
        common[f"Ws{i}"] = b(np.asarray(inputs["W_src"], np.float32)[i])
        common[f"Wd{i}"] = b(np.asarray(inputs["W_dst"], np.float32)[i])
        ar = np.broadcast_to(attn[i].reshape(1, HID), (P, HID))
        common[f"arep{i}"] = np.ascontiguousarray(ar).astype(bf16)

    in_maps = []
    for c in range(N_CORES):
        m = dict(common)
        m.update(per_core[c])
        in_maps.append(m)

    nc = build_nc(cfg)
    res = run_bass_kernel_spmd(nc, in_maps, core_ids=list(range(N_CORES)),
                               trace=trace)
    return np.asarray(res.results[0]["out"], np.float32), res


def kernel(**inputs) -> np.ndarray:
    out, _ = _run(inputs)
    return out


# revision 11
# speedup vs baseline: 1.1790x; 1.0417x over previous
"""Trainium2 Bass kernel for nn_DifferentPooling (GNN message passing).

Strategy (8 NeuronCores, SPMD):
  - Nodes padded to NP = 8*CHUNK and partitioned by node id across cores.
  - Edges partitioned by dst core; within a core, grouped into 128-node dst
    "windows". Aggregation (segment sum / softmax-sum) is done per window via
    one-hot selection matrices multiplied on the TensorEngine, accumulating in
    PSUM across the window's 128-edge tiles.
  - Feature rows are fetched with batched dma_gather (int16 indices, so the
    node table is split into two halves and each window's edges are split
    into lo/hi groups, each padded to 128-edge tiles).
  - After each layer, per-core node-feature chunks are AllGather'd so every
    core has the full table for the next layer's gathers.
  - GATv2 softmax uses exp(logit) without max subtraction (logits here are
    tiny), alpha = p / segsum(p) with a 1e-30 guard. The weighted aggregation
    uses sum_e w*eps = sum_e w*(fs+fd); since sum alpha = 1 per dst node,
    out = agg_eps/s - fd (fd subtracted only where the node has edges).
  - One-hot selection matrices (S_en and its transpose) are rebuilt on the
    Vector engine per window instead of being cached in DRAM.
  - Graph max-pooling: per-window masked-max segments interleaved into the
    last GAT layer -> small AllGather -> final max and a replicated fp32 MLP.

All biases in this problem are zeros by spec (fill="zeros"); they are not
applied on device.
"""

import sys

sys.path.insert(0, "/opt/trn_rl_repo")

import numpy as np
import ml_dtypes

bf16 = ml_dtypes.bfloat16

N_CORES = 8
P = 128  # window size / partition count
N_REAL = 50000
E_REAL = 500000
G = 64
HID = 128
HEADS = 8
DH = 16
OUT = 256


# ---------------------------------------------------------------------------
# Host-side preprocessing
# ---------------------------------------------------------------------------

def _wrap_idx(arr):
    """int idx array (len % 16 == 0) -> [128, len/16] int16 wrapped layout:
    idx i lives at [i % 16, i // 16], replicated across the 8 groups of 16
    partitions (one per Q7 core)."""
    a = np.asarray(arr, np.int16).reshape(-1, 16).T  # [16, cols]
    return np.tile(a, (8, 1))  # [128, cols]


def prep(src, dst, node2graph, nw_per_core):
    """Build per-core edge/window metadata. Returns (cfg, host arrays)."""
    NW = nw_per_core
    CHUNK = NW * P
    NP = N_CORES * CHUNK
    HALF = NP // 2
    N = len(node2graph)
    E = len(src)

    src = np.asarray(src, np.int64)
    dst = np.asarray(dst, np.int64)
    n2g = np.asarray(node2graph, np.int64)

    outdeg = np.zeros(NP, np.float32)
    np.add.at(outdeg, src, 1.0)
    indeg = np.zeros(NP, np.float32)
    np.add.at(indeg, dst, 1.0)
    ns = np.maximum(outdeg, 1.0) ** -0.5
    nd = np.maximum(indeg, 1.0) ** -0.5

    # sort edges by dst, bucket into windows
    order = np.argsort(dst, kind="stable")
    sdst = dst[order]
    ssrc = src[order]
    n_win_total = NP // P
    win_starts = np.searchsorted(sdst, np.arange(0, NP + 1, P))

    # per (global window): lo/hi edge lists sorted by src
    lo_lists, hi_lists = [], []
    max_lo = max_hi = 1
    for w in range(n_win_total):
        a, b = win_starts[w], win_starts[w + 1]
        es, ed = ssrc[a:b], sdst[a:b] - w * P
        m = es < HALF
        ordl = np.argsort(es[m], kind="stable")
        ordh = np.argsort(es[~m], kind="stable")
        lo_lists.append((es[m][ordl], ed[m][ordl]))
        hi_lists.append((es[~m][ordh] - HALF, ed[~m][ordh]))
        max_lo = max(max_lo, len(lo_lists[-1][0]))
        max_hi = max(max_hi, len(hi_lists[-1][0]))

    L = (max_lo + P - 1) // P
    H = (max_hi + P - 1) // P
    T = L + H

    # spans of SPAN_W windows (gather batching granularity)
    SPAN_W = 6 if NW >= 6 else 2
    spans = []
    w0 = 0
    while w0 < NW:
        spans.append((w0, min(SPAN_W, NW - w0)))
        w0 += SPAN_W

    per_core = []
    for c in range(N_CORES):
        idx_lo = np.zeros((NW, L * P), np.int64)
        dst_lo = np.full((NW, L * P), P, np.int64)  # sentinel 128
        idx_hi = np.zeros((NW, H * P), np.int64)
        dst_hi = np.full((NW, H * P), P, np.int64)
        for w in range(NW):
            el, dl = lo_lists[c * NW + w]
            eh, dh_ = hi_lists[c * NW + w]
            idx_lo[w, : len(el)] = el
            dst_lo[w, : len(dl)] = dl
            idx_hi[w, : len(eh)] = eh
            dst_hi[w, : len(dh_)] = dh_
        # dstloc: [NW*T, 128] -> transpose to [128, NW*T]; col w*T+t
        dstloc = np.concatenate(
            [dst_lo.reshape(NW, L, P), dst_hi.reshape(NW, H, P)], axis=1
        ).reshape(NW * T, P)
        fp8 = ml_dtypes.float8_e4m3fn
        dl_full = dstloc.reshape(NW, T, P)  # [w, t, e] dst values
        eye = np.arange(P)
        sden_h = (dl_full[:, :, :, None] == eye[None, None, None, :])
        # senw layout [w, e, t*n]
        sden_in = np.ascontiguousarray(
            sden_h.transpose(0, 2, 1, 3).reshape(NW, P, T * P)).astype(fp8)
        # snT layout [w, n, t*e]
        sntd_in = np.ascontiguousarray(
            sden_h.transpose(0, 3, 1, 2).reshape(NW, P, T * P)).astype(fp8)
        ndw = nd[c * CHUNK : (c + 1) * CHUNK].reshape(NW, P).T.copy()
        nsw = ns[c * CHUNK : (c + 1) * CHUNK].reshape(NW, P).T.copy()
        per_core.append(
            dict(
                idx_lo=_wrap_idx(idx_lo.reshape(-1)),
                idx_hi=_wrap_idx(idx_hi.reshape(-1)),
                dstloc=np.ascontiguousarray(dstloc.T).astype(bf16),
                dstflat=np.ascontiguousarray(
                    dstloc.reshape(NW, T * P)).astype(bf16),
                sdeni=sden_in, sntdi=sntd_in,
                ndw=np.ascontiguousarray(ndw, np.float32),
                ndnsw=np.ascontiguousarray(ndw * nsw, np.float32),
            )
        )

    # pooling segments per core: runs of equal graph id inside each window
    n2g_pad = np.full(NP, -1, np.int64)
    n2g_pad[:N] = n2g
    seg_all = []  # per core: list of (w, j0, j1, g)
    KSEG = 1
    for c in range(N_CORES):
        segs = []
        for w in range(NW):
            ids = n2g_pad[c * CHUNK + w * P : c * CHUNK + (w + 1) * P]
            j = 0
            wsegs = []
            while j < P:
                g = ids[j]
                k = j
                while k < P and ids[k] == g:
                    k += 1
                if g >= 0:
                    wsegs.append((j, k, int(g)))
                j = k
            KSEG = max(KSEG, len(wsegs))
            segs.append(wsegs)
        seg_all.append(segs)

    BIG = np.float32(1e30)
    NSEG = NW * KSEG
    for c in range(N_CORES):
        maskvec = np.full((NW, KSEG, P), -BIG, np.float32)
        gmask = np.full((G, NSEG), -BIG, np.float32)
        for w in range(NW):
            for k, (j0, j1, g) in enumerate(seg_all[c][w]):
                maskvec[w, k, j0:j1] = BIG
                gmask[g, w * KSEG + k] = BIG
        per_core[c]["poolmask"] = maskvec.reshape(NW, KSEG * P).astype(bf16)
        per_core[c]["gmask"] = gmask.astype(bf16)

    cfg = dict(NW=NW, CHUNK=CHUNK, NP=NP, HALF=HALF, L=L, H=H, T=T,
               spans=spans, KSEG=KSEG)
    return cfg, per_core, ns, nd


# ---------------------------------------------------------------------------
# Bass kernel builder
# ---------------------------------------------------------------------------

def build_nc(cfg):
    import concourse.bacc as bacc
    import concourse.bass as bass
    import concourse.mybir as mybir
    import concourse.tile as tile
    from concourse.masks import make_identity

    NW, CHUNK, NP, HALF = cfg["NW"], cfg["CHUNK"], cfg["NP"], cfg["HALF"]
    L, H, T, spans, KSEG = cfg["L"], cfg["H"], cfg["T"], cfg["spans"], cfg["KSEG"]
    FP = mybir.dt.float32
    BF = mybir.dt.bfloat16
    AO = mybir.AluOpType
    AFT = mybir.ActivationFunctionType
    GT = 6  # edge tiles per PSUM group in GAT layers

    nc = bacc.Bacc("TRN2", target_bir_lowering=False, debug=False,
                   num_devices=N_CORES)

    def din(name, shape, dt=BF):
        return nc.dram_tensor(name, shape, dt, kind="ExternalInput")

    table0 = din("table0", [NP, P])
    Wgc = [din(f"Wgc{i}", [P, P]) for i in range(2)]
    Ws = [din(f"Ws{i}", [P, P]) for i in range(3)]
    Wd = [din(f"Wd{i}", [P, P]) for i in range(3)]
    arep = [din(f"arep{i}", [P, P]) for i in range(3)]
    Wc1 = din("Wc1", [P, P], FP)
    Wc2 = din("Wc2", [P, 64], FP)
    Wc3 = din("Wc3", [64, OUT], FP)
    idx_lo = din("idx_lo", [P, NW * L * P // 16], mybir.dt.int16)
    idx_hi = din("idx_hi", [P, NW * H * P // 16], mybir.dt.int16)
    dstloc = din("dstloc", [P, NW * T])
    dstflat = din("dstflat", [NW, T * P])
    sdeni = din("sdeni", [NW, P, T * P], mybir.dt.float8e4)
    sntdi = din("sntdi", [NW, P, T * P], mybir.dt.float8e4)
    ndw = din("ndw", [P, NW], FP)
    ndnsw = din("ndnsw", [P, NW], FP)
    poolmask = din("poolmask", [NW, KSEG * P])
    gmask = din("gmask", [G, NW * KSEG])

    out_ext = nc.dram_tensor("out", [G, OUT], FP, kind="ExternalOutput")

    # internal DRAM
    agin = [nc.dram_tensor(f"agin{i}", [CHUNK, P], BF) for i in range(4)]
    tables = [table0] + [
        nc.dram_tensor(f"table{i+1}", [NP, P], BF, addr_space="Shared")
        for i in range(4)
    ]
    hgpart = nc.dram_tensor("hgpart", [P, G], FP)
    hgall = nc.dram_tensor("hgall", [N_CORES * P, G], FP, addr_space="Shared")

    RG = [list(range(N_CORES))]

    with tile.TileContext(nc) as tc:
        import contextlib

        ctx = contextlib.ExitStack()
        with ctx:
            const_pool = ctx.enter_context(tc.tile_pool(name="const", bufs=1))
            stg_pool = ctx.enter_context(tc.tile_pool(name="stg", bufs=2))
            sb_pool = ctx.enter_context(tc.tile_pool(name="sb", bufs=3))
            chunk_pool = ctx.enter_context(tc.tile_pool(name="chunk", bufs=1))
            ps_pool = ctx.enter_context(
                tc.tile_pool(name="ps", bufs=2, space="PSUM")
            )
            agg_pool = ctx.enter_context(
                tc.tile_pool(name="agg", bufs=2, space="PSUM")
            )
            mini_ps = ctx.enter_context(
                tc.tile_pool(name="minips", bufs=2, space="PSUM")
            )

            # --- constants in SBUF ---
            ident_bf = const_pool.tile([P, P], BF, tag="identbf")
            make_identity(nc, ident_bf[:])
            ident_f = const_pool.tile([P, P], FP, tag="identf")
            make_identity(nc, ident_f[:])
            iota_f = const_pool.tile([P, P], BF, tag="iota")
            iota_i = const_pool.tile([P, P], mybir.dt.int32, tag="iotai")
            nc.gpsimd.iota(iota_i[:], pattern=[[1, P]], base=0,
                           channel_multiplier=0)
            nc.vector.tensor_copy(iota_f[:], iota_i[:])
            iotap_f = const_pool.tile([P, 1], FP, tag="iotap")
            iotap_i = const_pool.tile([P, 1], mybir.dt.int32, tag="iotapi")
            nc.gpsimd.iota(iotap_i[:], pattern=[[0, 1]], base=0,
                           channel_multiplier=1)
            nc.vector.tensor_copy(iotap_f[:], iotap_i[:])

            def load_const(h, shape, dt=BF, tag=None):
                t = const_pool.tile(shape, dt, tag=tag or h.name)
                nc.sync.dma_start(t[:], h[:])
                return t

            Wgc_sb = [load_const(w, [P, P]) for w in Wgc]
            Ws_sb = [load_const(w, [P, P]) for w in Ws]
            Wd_sb = [load_const(w, [P, P]) for w in Wd]
            arep_sb = [load_const(w, [P, P]) for w in arep]
            dstloc_sb = load_const(dstloc, [P, NW * T])
            ndw_sb = load_const(ndw, [P, NW], FP)
            ndnsw_sb = load_const(ndnsw, [P, NW], FP)
            idxlo_sb = load_const(idx_lo, [P, NW * L * P // 16], mybir.dt.int16)
            idxhi_sb = load_const(idx_hi, [P, NW * H * P // 16], mybir.dt.int16)

            def s_en_build_window(w):
                """Build S_en for all T tiles of window w: [128, T, 128]
                ([e, tile, n]) on the vector engine."""
                senw = sb_pool.tile([P, T, P], F8, tag="senw", bufs=2)
                nc.vector.tensor_tensor(
                    out=senw[:],
                    in0=dstloc_sb[:, w * T : (w + 1) * T]
                    .unsqueeze(2)
                    .to_broadcast([P, T, P]),
                    in1=iota_f[:].unsqueeze(1).to_broadcast([P, T, P]),
                    op=AO.is_equal,
                )
                return senw

            SLOAD_W = 2  # windows per S_en reload DMA

            def s_en_load(w0, nwin):
                sload = sb_pool.tile([P, SLOAD_W, T * P], F8, tag="sload",
                                     bufs=2)
                nc.sync.dma_start(
                    sload[:, :nwin, :],
                    sdeni[w0 : w0 + nwin].rearrange("w p f -> p w f"),
                )
                return sload

            F8 = mybir.dt.float8e4  # noqa: N806

            def snt4_build(w, n4):
                """Load transposed one-hots [node, e] for windows w..w+n4."""
                snT4 = sb_pool.tile([P, 4, T, P], F8, tag="snT4", bufs=2,
                                    name="snT4")
                nc.sync.dma_start(
                    snT4[:, :n4, :, :].rearrange("p w t f -> p w (t f)"),
                    sntdi[w : w + n4].rearrange("w p f -> p w f"),
                )
                return snT4

            def gather_span(table_l, w0, nw, transpose):
                """Gather all edges of windows [w0, w0+nw). Returns
                (stg_lo, stg_hi): transpose -> [128, 1, n] column tiles,
                else [128, ntiles, 128] row tiles."""
                nlo, nhi = nw * L * P, nw * H * P
                outs = []
                for which, n, idx_sb, colpos in (
                    ("lo", nlo, idxlo_sb, w0 * L * P),
                    ("hi", nhi, idxhi_sb, w0 * H * P),
                ):
                    half = table_l[0:HALF, :] if which == "lo" else table_l[HALF:NP, :]
                    if transpose:
                        t = stg_pool.tile([P, 1, n], BF, tag=f"stg{which}")
                        o = t[:, :, :]
                    else:
                        t = stg_pool.tile([P, n // P, P], BF, tag=f"stg{which}")
                        o = t[:, :, :]
                    nc.gpsimd.dma_gather(
                        o,
                        half,
                        idx_sb[:, colpos // 16 : (colpos + n) // 16],
                        n,
                        n,
                        P,
                        transpose=transpose,
                        single_packet=False,
                    )
                    outs.append(t)
                return outs

            # =========================================================
            # GraphConv layers
            # =========================================================
            def gc_layer(li, table_l, W_sb, agin_out, scale_sb, htag):
                hnew = chunk_pool.tile([P, NW, P], BF, tag=htag)
                for (w0, nw) in spans:
                    stg_lo, stg_hi = gather_span(table_l, w0, nw, False)
                    for wr in range(nw):
                        w = w0 + wr
                        senw = s_en_build_window(w)
                        aggT_full = agg_pool.tile([P, P + 16], FP, tag="agg",
                                                  name="aggT")
                        aggT = aggT_full[:, :P]
                        for t in range(T):
                            if t < L:
                                lhs = stg_lo[:, wr * L + t, :]
                            else:
                                lhs = stg_hi[:, wr * H + (t - L), :]
                            nc.tensor.matmul(
                                out=aggT[:],
                                lhsT=lhs,
                                rhs=senw[:, t, :],
                                start=(t == 0),
                                stop=(t == T - 1),
                            )
                        aggT_sb = sb_pool.tile([P, P], BF, tag="aggTsb")
                        nc.scalar.copy(aggT_sb[:], aggT[:])
                        op = mini_ps.tile([P, P], FP, tag="mini")
                        nc.tensor.matmul(out=op[:], lhsT=aggT_sb[:], rhs=W_sb[:],
                                         start=True, stop=True)
                        nc.scalar.activation(
                            hnew[:, w, :], op[:], AFT.Relu,
                            scale=scale_sb[:, w : w + 1],
                        )
                nc.sync.dma_start(
                    agin_out[:].rearrange("(w p) f -> p w f", p=P), hnew[:]
                )
                return hnew

            # =========================================================
            # GATv2 layers
            # =========================================================
            def fdw_prep(h_tile, Wd_l, tag):
                """fd = h @ Wd per window, from the local chunk tile."""
                fdw = chunk_pool.tile([P, NW, P], BF, tag=tag)
                for w in range(NW):
                    tp = mini_ps.tile([P, P], BF, tag="mini")
                    nc.tensor.transpose(tp[:], h_tile[:, w, :], ident_bf[:])
                    hwT = sb_pool.tile([P, P], BF, tag="hwTsb")
                    nc.scalar.copy(hwT[:], tp[:])
                    fp = mini_ps.tile([P, P], FP, tag="mini")
                    nc.tensor.matmul(out=fp[:], lhsT=hwT[:], rhs=Wd_l[:],
                                     start=True, stop=True)
                    nc.scalar.copy(fdw[:, w, :], fp[:])
                return fdw

            def gat_layer(li, table_l, hprev, fdw, Ws_l, arep_l, agin_out,
                          htag, pool_state=None):
                hnew = chunk_pool.tile([P, NW, P], BF, tag=htag)
                for (w0, nw) in spans:
                    stg_lo, stg_hi = gather_span(table_l, w0, nw, True)
                    for wr in range(nw):
                        w = w0 + wr
                        if wr % 4 == 0:
                            snT4 = snt4_build(w, min(4, nw - wr))
                        if wr % SLOAD_W == 0:
                            sload = s_en_load(w, min(SLOAD_W, nw - wr))
                        snTw = snT4[:, wr % 4]  # [P, T, P]
                        swi = wr % SLOAD_W
                        agg = agg_pool.tile([P, P + 16], FP, tag="agg")
                        for g0 in range(0, T, GT):
                            gn = min(GT, T - g0)
                            eps = ps_pool.tile([P, GT * P], FP, tag="eps")
                            for k in range(gn):
                                t = g0 + k
                                if t < L:
                                    col = (wr * L + t) * P
                                    hsT = stg_lo[:, 0, col : col + P]
                                else:
                                    col = (wr * H + (t - L)) * P
                                    hsT = stg_hi[:, 0, col : col + P]
                                sl = slice(k * P, (k + 1) * P)
                                nc.tensor.matmul(out=eps[:, sl], lhsT=hsT,
                                                 rhs=Ws_l[:], start=True,
                                                 stop=False)
                                nc.tensor.matmul(out=eps[:, sl],
                                                 lhsT=snTw[:, t, :],
                                                 rhs=fdw[:, w, :], start=False,
                                                 stop=True)
                            epsv = eps[:, : gn * P].rearrange(
                                "p (a b) -> p a b", b=P
                            )
                            elr = sb_pool.tile([P, GT, P], BF, tag="elr",
                                               bufs=2)
                            nc.scalar.activation(elr[:, :gn, :], epsv,
                                                 AFT.Prelu, alpha=0.2)
                            eps_sb = sb_pool.tile([P, GT, P], BF, tag="epssb",
                                                  bufs=2)
                            nc.scalar.copy(eps_sb[:, :gn, :], epsv)
                            prod = sb_pool.tile([P, GT, P], BF, tag="prod",
                                                bufs=2)
                            nc.vector.tensor_tensor(
                                out=prod[:, :gn, :], in0=elr[:, :gn, :],
                                in1=arep_l[:].unsqueeze(1).to_broadcast(
                                    [P, gn, P]
                                ),
                                op=AO.mult,
                            )
                            pv = prod[:, :gn, :].rearrange(
                                "p a (h d) -> p (a h) d", d=DH
                            )
                            fold8 = sb_pool.tile([P, GT * HEADS, 8], BF,
                                                 tag="fold8", bufs=2)
                            nc.vector.tensor_tensor(
                                out=fold8[:, : gn * HEADS, :],
                                in0=pv[:, :, 0:8], in1=pv[:, :, 8:16],
                                op=AO.add,
                            )
                            logit = sb_pool.tile([P, GT * HEADS], FP,
                                                 tag="logit")
                            nc.vector.tensor_reduce(
                                out=logit[:, : gn * HEADS],
                                in_=fold8[:, : gn * HEADS, :],
                                axis=mybir.AxisListType.X,
                                op=AO.add,
                            )
                            wf = sb_pool.tile([P, GT, P + 8], BF, tag="wf",
                                              bufs=2)
                            nc.scalar.activation(
                                wf[:, :gn, P : P + 8],
                                logit[:, : gn * HEADS].rearrange(
                                    "p (a b) -> p a b", b=HEADS
                                ),
                                AFT.Exp,
                            )
                            nc.vector.tensor_tensor(
                                out=wf[:, :gn, 0:P].rearrange(
                                    "p a (h d) -> p a h d", d=DH
                                ),
                                in0=eps_sb[:, :gn, :].rearrange(
                                    "p a (h d) -> p a h d", d=DH
                                ),
                                in1=wf[:, :gn, P : P + 8]
                                .unsqueeze(3)
                                .to_broadcast([P, gn, HEADS, DH]),
                                op=AO.mult,
                            )
                            for k in range(gn):
                                t = g0 + k
                                nc.tensor.matmul(
                                    out=agg[:, : P + 8],
                                    lhsT=sload[:, swi, t * P : (t + 1) * P],
                                    rhs=wf[:, k, :],
                                    start=(t == 0),
                                    stop=(t == T - 1),
                                )
                        # ---- window flush ----
                        sguard = sb_pool.tile([P, 8], FP, tag="sguard")
                        nc.vector.tensor_scalar_max(
                            sguard[:], agg[:, P : P + 8], 1e-30
                        )
                        rec = sb_pool.tile([P, 8], FP, tag="rec")
                        nc.vector.reciprocal(rec[:], sguard[:])
                        ind = sb_pool.tile([P, 8], BF, tag="ind")
                        nc.vector.tensor_scalar(
                            out=ind[:], in0=agg[:, P : P + 8],
                            scalar1=1e-20, scalar2=None, op0=AO.is_gt,
                        )
                        fdind = sb_pool.tile([P, P], BF, tag="fdind")
                        nc.gpsimd.tensor_tensor(
                            out=fdind[:].rearrange("p (h d) -> p h d", d=DH),
                            in0=fdw[:, w, :].rearrange("p (h d) -> p h d", d=DH),
                            in1=ind[:].unsqueeze(2).to_broadcast([P, HEADS, DH]),
                            op=AO.mult,
                        )
                        hmfd = sb_pool.tile([P, P], BF, tag="hmfd")
                        nc.gpsimd.tensor_tensor(
                            out=hmfd[:], in0=hprev[:, w, :], in1=fdind[:],
                            op=AO.subtract,
                        )
                        o2 = sb_pool.tile([P, P], FP, tag="o2")
                        nc.vector.tensor_tensor(
                            out=o2[:].rearrange("p (h d) -> p h d", d=DH),
                            in0=agg[:, 0:P].rearrange("p (h d) -> p h d", d=DH),
                            in1=rec[:].unsqueeze(2).to_broadcast([P, HEADS, DH]),
                            op=AO.mult,
                        )
                        o3 = sb_pool.tile([P, P], FP, tag="o3")
                        nc.gpsimd.tensor_tensor(
                            out=o3[:], in0=o2[:], in1=hmfd[:], op=AO.add
                        )
                        nc.scalar.activation(hnew[:, w, :], o3[:], AFT.Relu)
                        if pool_state is not None:
                            pool_window(pool_state, hnew, w)
                if agin_out is not None:
                    nc.sync.dma_start(
                        agin_out[:].rearrange("(w p) f -> p w f", p=P), hnew[:]
                    )
                return hnew

            # =========================================================
            # pooling (interleaved into the last GAT layer)
            # =========================================================
            def pool_window(st, hnew, w):
                tp = mini_ps.tile([P, P], BF, tag="mini")
                nc.tensor.transpose(tp[:], hnew[:, w, :], ident_bf[:])
                h5t = sb_pool.tile([P, P], BF, tag="h5t")
                nc.vector.tensor_copy(h5t[:], tp[:])
                if w % 8 == 0:
                    nw8 = min(8, NW - w)
                    st["pmask"] = sb_pool.tile(
                        [P, 8, KSEG * P], BF, tag="pmask8", bufs=1,
                        name="pmask_rep8"
                    )
                    nc.sync.dma_start(
                        st["pmask"][:, :nw8, :],
                        poolmask[w : w + nw8, :]
                        .unsqueeze(0)
                        .to_broadcast([P, nw8, KSEG * P]),
                    )
                msk = sb_pool.tile([P, KSEG, P], BF, tag="msk")
                nc.vector.tensor_tensor(
                    out=msk[:],
                    in0=h5t[:].unsqueeze(1).to_broadcast([P, KSEG, P]),
                    in1=st["pmask"][:, w % 8].rearrange(
                        "p (k b) -> p k b", b=P
                    ),
                    op=AO.min,
                )
                nc.vector.tensor_reduce(
                    out=st["stag"][:, w * KSEG : (w + 1) * KSEG],
                    in_=msk[:],
                    axis=mybir.AxisListType.X,
                    op=AO.max,
                )

            # =========================================================
            # forward pass
            # =========================================================
            h1 = gc_layer(0, tables[0], Wgc_sb[0], agin[0], ndnsw_sb, "hA")
            nc.gpsimd.collective_compute(
                "AllGather", AO.bypass, replica_groups=RG,
                ins=[agin[0].ap().opt()], outs=[tables[1].ap().opt()],
            )
            h2 = gc_layer(1, tables[1], Wgc_sb[1], agin[1], ndw_sb, "hB")
            fdw0 = fdw_prep(h2, Wd_sb[0], "fdwA")
            nc.gpsimd.collective_compute(
                "AllGather", AO.bypass, replica_groups=RG,
                ins=[agin[1].ap().opt()], outs=[tables[2].ap().opt()],
            )
            h3 = gat_layer(0, tables[2], h2, fdw0, Ws_sb[0], arep_sb[0],
                           agin[2], "hA")
            fdw1 = fdw_prep(h3, Wd_sb[1], "fdwB")
            nc.gpsimd.collective_compute(
                "AllGather", AO.bypass, replica_groups=RG,
                ins=[agin[2].ap().opt()], outs=[tables[3].ap().opt()],
            )
            h4 = gat_layer(1, tables[3], h3, fdw1, Ws_sb[1], arep_sb[1],
                           agin[3], "hB")
            fdw2 = fdw_prep(h4, Wd_sb[2], "fdwA")
            nc.gpsimd.collective_compute(
                "AllGather", AO.bypass, replica_groups=RG,
                ins=[agin[3].ap().opt()], outs=[tables[4].ap().opt()],
            )
            NSEG = NW * KSEG
            stag_t = chunk_pool.tile([P, NSEG], FP, tag="stag")
            pool_state = dict(stag=stag_t, pmask=None)
            gat_layer(2, tables[4], h4, fdw2, Ws_sb[2], arep_sb[2],
                      None, "hA", pool_state=pool_state)

            # =========================================================
            # graph-level max + MLP (replicated)
            # =========================================================
            stag = pool_state["stag"]
            gmask_all = sb_pool.tile([P, G, NSEG], BF, tag="gmaskall", bufs=1)
            nc.sync.dma_start(
                gmask_all[:],
                gmask[:].unsqueeze(0).to_broadcast([P, G, NSEG]),
            )
            gm = sb_pool.tile([P, G, NSEG], BF, tag="gm", bufs=1)
            nc.vector.tensor_tensor(
                out=gm[:],
                in0=stag[:, :NSEG].unsqueeze(1).to_broadcast([P, G, NSEG]),
                in1=gmask_all[:],
                op=AO.min,
            )
            hgT_part = sb_pool.tile([P, G], FP, tag="hgT_part")
            nc.vector.tensor_reduce(
                out=hgT_part[:], in_=gm[:],
                axis=mybir.AxisListType.X, op=AO.max,
            )
            nc.sync.dma_start(hgpart[:], hgT_part[:])
            nc.gpsimd.collective_compute(
                "AllGather", AO.bypass, replica_groups=RG,
                ins=[hgpart.ap().opt()], outs=[hgall.ap().opt()],
            )
            # final max over ranks: hgall rows = (r p)
            hgl = sb_pool.tile([P, N_CORES * G], FP, tag="hgl")
            nc.sync.dma_start(
                hgl[:].rearrange("p (r g) -> p r g", g=G),
                hgall[:].rearrange("(r p) g -> p r g", p=P),
            )
            hgT = sb_pool.tile([P, G], FP, tag="hgT")
            nc.vector.tensor_reduce(
                out=hgT[:],
                in_=hgl[:].rearrange("p (r g) -> p g r", g=G),
                axis=mybir.AxisListType.X, op=AO.max,
            )

            Wc1_sb = load_const(Wc1, [P, P], FP)
            Wc2_sb = load_const(Wc2, [P, 64], FP)
            Wc3_sb = load_const(Wc3, [64, OUT], FP)

            z1p = mini_ps.tile([G, P], FP, tag="mini")
            nc.tensor.matmul(out=z1p[:], lhsT=hgT[:], rhs=Wc1_sb[:],
                             start=True, stop=True)
            z1 = sb_pool.tile([G, P], FP, tag="z1")
            nc.scalar.activation(z1[:], z1p[:], AFT.Relu)
            z1Tp = mini_ps.tile([P, G], FP, tag="mini")
            nc.tensor.transpose(z1Tp[:], z1[:], ident_f[:G, :G])
            z1T = sb_pool.tile([P, G], FP, tag="z1T")
            nc.scalar.copy(z1T[:], z1Tp[:])
            z2p = mini_ps.tile([G, 64], FP, tag="mini")
            nc.tensor.matmul(out=z2p[:], lhsT=z1T[:], rhs=Wc2_sb[:],
                             start=True, stop=True)
            z2 = sb_pool.tile([G, 64], FP, tag="z2")
            nc.scalar.activation(z2[:], z2p[:], AFT.Relu)
            z2Tp = mini_ps.tile([64, G], FP, tag="mini")
            nc.tensor.transpose(z2Tp[:], z2[:], ident_f[:G, :G])
            z2T = sb_pool.tile([64, G], FP, tag="z2T")
            nc.scalar.copy(z2T[:], z2Tp[:])
            z3p = mini_ps.tile([G, OUT], FP, tag="mini")
            nc.tensor.matmul(out=z3p[:], lhsT=z2T[:], rhs=Wc3_sb[:],
                             start=True, stop=True)
            z3 = sb_pool.tile([G, OUT], FP, tag="z3")
            nc.scalar.copy(z3[:], z3p[:])
            nc.sync.dma_start(out_ext[:], z3[:])

    nc.compile()
    return nc


# ---------------------------------------------------------------------------
# Entry point
# ---------------------------------------------------------------------------

def _run(inputs, nw_per_core=49, trace=False):
    from concourse.bass_utils import run_bass_kernel_spmd

    src = np.asarray(inputs["src"])
    dst = np.asarray(inputs["dst"])
    n2g = np.asarray(inputs["node2graph"])
    feat = np.asarray(inputs["feature"], np.float32)

    cfg, per_core, ns, nd = prep(src, dst, n2g, nw_per_core)
    NP = cfg["NP"]

    featp = np.zeros((NP, P), np.float32)
    featp[: feat.shape[0]] = feat
    featp *= ns[:, None]
    table0 = featp.astype(bf16)

    def b(x):
        return np.ascontiguousarray(np.asarray(x, np.float32).astype(bf16))

    common = dict(
        table0=table0,
        Wgc0=b(inputs["W_gc1"]), Wgc1=b(inputs["W_gc2"]),
        Wc1=np.ascontiguousarray(np.asarray(inputs["Wc1"], np.float32)),
        Wc2=np.ascontiguousarray(np.asarray(inputs["Wc2"], np.float32)),
        Wc3=np.ascontiguousarray(np.asarray(inputs["Wc3"], np.float32)),
    )
    attn = np.asarray(inputs["attn"], np.float32)
    for i in range(3):
        common[f"Ws{i}"] = b(np.asarray(inputs["W_src"], np.float32)[i])
        common[f"Wd{i}"] = b(np.asarray(inputs["W_dst"], np.float32)[i])
        ar = np.broadcast_to(attn[i].reshape(1, HID), (P, HID))
        common[f"arep{i}"] = np.ascontiguousarray(ar).astype(bf16)

    in_maps = []
    for c in range(N_CORES):
        m = dict(common)
        m.update(per_core[c])
        in_maps.append(m)

    nc = build_nc(cfg)
    res = run_bass_kernel_spmd(nc, in_maps, core_ids=list(range(N_CORES)),
                               trace=trace)
    return np.asarray(res.results[0]["out"], np.float32), res


def kernel(**inputs) -> np.ndarray:
    out, _ = _run(inputs)
    return out
